# revision 2
# baseline (speedup 1.0000x reference)
# Trainium2 Bass kernel for nn_EqPropNetwork (equilibrium-propagation relaxation).
#
# Math (per reference.py):
#   c_h = x @ W1 + b1                                  [B, HID]  (constant over steps)
#   repeat T times:
#     psi = y @ W2.T ; phi = h @ W2
#     h'  = clip(0.5*h + 0.5*c_h + 0.5*psi, 0, 1)
#     y'  = clip(0.25*y + 0.5*phi + 0.5*b2 + 0.25*onehot(target), 0, 1)
#   out = concat(h, y)                                  [B, HID+OUT]
#
# Mapping (per core, B_loc = 4096, pure data parallel over 8 cores):
#   Feature-major state: partition = feature, free = batch; 4 chunks x 4
#   batch-blocks of [128, 1024].  Engine-balanced routes per chunk:
#   * chunks 0,1 ("R1"): state = h' (post-clip).  PE identity streams BOTH
#     h' and c_h (PSUM u = 0.5h' + 0.5c_h + 0.5psi); ACT relu evacuates
#     PSUM; DVE finishes with a 4x-mode min/max.  (PE-side add is cheaper
#     than a DVE tensor_tensor add.)
#   * chunks 2,3: state = s := h + c_h.  PSUM u = 0.5s + 0.5psi.  Most
#     blocks: ACT relu + one DVE scalar_tensor_tensor (min,add); one pair
#     runs DVE-only (clip from PSUM + 2x tensor_tensor add) to offload ACT.
#   * y-update rides PE: py = 0.5*(s@W2) + 0.25*y (diag-tiled I10 matmuls)
#     + dbar (identity matmul streaming dbar2 = 2*dbar), then a single DVE
#     clip.  dbar folds onehot/4 + b2/2 - 0.5*(c_h@W2) over the s-chunks.
#   * psi matmuls: per-strip LDWEIGHTS (rows 32m..32m+10 only) and block
#     order rotates the batch-block m so adjacent psi matmuls land on
#     distinct PE row groups and overlap.  phi matmuls are 4-way
#     col-group packed.
import sys

import numpy as np

if "/opt/trn_rl_repo" not in sys.path:
    sys.path.insert(0, "/opt/trn_rl_repo")

N_CORES = 8
B, IN, HID, OUT = 32768, 784, 512, 10
BLOC = B // N_CORES  # 4096
NBLK = BLOC // 1024  # 4 batch blocks of 1024
KIN = 7              # IN chunks of 112
KC = IN // KIN       # 112
HCH = HID // 128     # 4 hidden chunks

R1C = (0, 1)         # chunks with h'-state (PE adds c_h)
SC = (2, 3)          # chunks with s-state

# packed fp16 const tile column offsets
C_HALFI = 0          # [128, 128] 0.5*I128
C_W2TR = 128         # 4 x [128, 128] W2T replicated, pre-scaled 0.5
C_I10Q = 640         # [128, 10] 0.25*I10 at 4 row offsets
C_R2 = 650           # [128, 128] rep: R2[32j, 32j+i]=1 (i<10) -> bcast block rows
CF16_W = 778

# per-chunk block (batch-block m) orders: adjacent blocks get distinct m
# (psi row-group overlap) and pair halves land contiguously (2048-col DVE ops)
CHUNK_MS = {0: (0, 1, 2, 3), 1: (1, 0, 3, 2), 2: (3, 2, 1, 0), 3: (2, 3, 0, 1)}

_BUILT = {}


def _build(T):
    import concourse.bass as bass
    from concourse import bacc, mybir
    from concourse.tile import TileContext

    f32 = mybir.dt.float32
    f16 = mybir.dt.float16
    i32 = mybir.dt.int32
    Alu = mybir.AluOpType
    Act = mybir.ActivationFunctionType

    nc = bacc.Bacc("TRN2", target_bir_lowering=False)

    xT = nc.declare_dram_parameter("xT", [IN, BLOC], f32, isOutput=False)
    hT = nc.declare_dram_parameter("hT", [HID, BLOC], f32, isOutput=False)
    yT = nc.declare_dram_parameter("yT", [OUT, BLOC], f32, isOutput=False)
    W1 = nc.declare_dram_parameter("W1", [IN, HID], f32, isOutput=False)
    W2 = nc.declare_dram_parameter("W2", [HID, OUT], f32, isOutput=False)
    b1 = nc.declare_dram_parameter("b1", [HID, 1], f32, isOutput=False)
    b2 = nc.declare_dram_parameter("b2", [OUT, 1], f32, isOutput=False)
    tgt = nc.declare_dram_parameter("tgt", [BLOC, 2], i32, isOutput=False)
    cst16 = nc.declare_dram_parameter("cst16", [128, CF16_W], f16, isOutput=False)
    cst32 = nc.declare_dram_parameter("cst32", [128, 8], f32, isOutput=False)

    hT_out = nc.declare_dram_parameter("hT_out", [HID, BLOC], f32, isOutput=True)
    yT_out = nc.declare_dram_parameter("yT_out", [OUT, BLOC], f32, isOutput=True)

    with TileContext(nc) as tc:
        with (
            tc.tile_pool(name="const", bufs=1) as constp,
            tc.tile_pool(name="ch", bufs=1) as chp,
            tc.tile_pool(name="state", bufs=2) as sp,
            tc.tile_pool(name="ypool", bufs=2) as yp,
        ):
            cf16 = constp.tile([128, CF16_W], f16, tag="cf16", name="cf16")
            cf32 = constp.tile([128, 8], f32, tag="cf32", name="cf32")
            dbar2 = constp.tile([128, 1024], f16, tag="dbar2", name="dbar2")
            cb1 = constp.tile([128, HCH], f32, tag="cb1", name="cb1")
            nc.sync.dma_start(out=cf16[:], in_=cst16[:])
            nc.sync.dma_start(out=cf32[:], in_=cst32[:])
            nc.sync.dma_start(
                out=cb1.rearrange("p (c o) -> p c o", c=HCH),
                in_=b1.rearrange("(c p) o -> p c o", c=HCH),
            )

            halfI_t = cf16[:, C_HALFI:C_HALFI + 128]
            W2Tr = [cf16[:, C_W2TR + 128 * c:C_W2TR + 128 * (c + 1)]
                    for c in range(HCH)]
            I10q_t = cf16[:, C_I10Q:C_I10Q + OUT]
            idxf_t = cf32[:, 0:1]
            b1c = [cb1[:, c:c + 1] for c in range(HCH)]
            R2_t = cf16[:, C_R2:C_R2 + 128]

            ch = chp.tile([128, HCH * BLOC], f16, tag="ch", name="ch")
            chv = [ch[:, BLOC * c:BLOC * (c + 1)] for c in range(HCH)]

            # ---------- setup phase A: c_h = x@W1 + b1 ----------
            # x loaded in batch stripes of 512: a stripe's x@W1 matmuls start
            # as soon as its 7 IN-chunks land (DMA overlaps PE).
            with (
                tc.tile_pool(name="x16p", bufs=1) as x16p,
                tc.tile_pool(name="stage", bufs=8) as stagep,
                tc.tile_pool(name="mst", bufs=3) as mstp,
                tc.tile_pool(name="spsum", bufs=4, space="PSUM") as spsum,
            ):
                w1_16 = x16p.tile([128, KIN * HID], f16, tag="w1_16", name="w1_16")
                for k in range(KIN):
                    st = mstp.tile([128, HID], f32, tag="mst", name="mst")
                    nc.sync.dma_start(out=st[:KC, :], in_=W1[KC * k:KC * (k + 1), :])
                    nc.vector.tensor_copy(
                        w1_16[:KC, HID * k:HID * (k + 1)], st[:KC, :]
                    )
                x16 = x16p.tile([128, KIN * BLOC], f16, tag="x16", name="x16")
                for blk in range(BLOC // 512):
                    bsl = slice(512 * blk, 512 * (blk + 1))
                    for k in range(KIN):
                        st = stagep.tile([128, 512], f32, tag="stage", name="stage")
                        nc.sync.dma_start(
                            out=st[:KC, :], in_=xT[KC * k:KC * (k + 1), bsl]
                        )
                        nc.vector.tensor_copy(
                            x16[:KC, BLOC * k + 512 * blk:BLOC * k + 512 * (blk + 1)],
                            st[:KC, :],
                        )
                    for c in range(HCH):
                        ps = spsum.tile([128, 512], f32, tag="spsum", name="spsum")
                        for k in range(KIN):
                            nc.tensor.matmul(
                                ps[:],
                                w1_16[:KC, HID * k + 128 * c:HID * k + 128 * (c + 1)],
                                x16[:KC, BLOC * k + 512 * blk:BLOC * k + 512 * (blk + 1)],
                                start=(k == 0),
                                stop=(k == KIN - 1),
                                tile_position=(0, 0),
                            )
                        nc.scalar.activation(
                            chv[c][:, 512 * blk:512 * (blk + 1)],
                            ps[:],
                            Act.Identity,
                            bias=b1c[c],
                            scale=1.0,
                        )

            # ---------- setup phase B: W2 forms, dbar2, y0, state0 ----------
            with (
                tc.tile_pool(name="stage2", bufs=2) as stage2p,
                tc.tile_pool(name="mst2", bufs=3) as mst2p,
                tc.tile_pool(name="spsum2", bufs=4, space="PSUM") as spsum2,
            ):
                # W2c = 0.5*W2 chunks [128, 4*10] (phi stationaries)
                w2c16 = mst2p.tile([128, HCH * OUT], f16, tag="w2c16",
                                   name="w2c16", bufs=1)
                st = mst2p.tile([128, HCH * OUT], f32, tag="mst2", name="mst2")
                nc.sync.dma_start(
                    out=st.rearrange("p (c i) -> p c i", c=HCH),
                    in_=W2.rearrange("(c p) i -> p c i", c=HCH),
                )
                nc.vector.tensor_scalar_mul(w2c16[:], st[:], 0.5)
                W2c = [w2c16[:, OUT * c:OUT * (c + 1)] for c in range(HCH)]

                # W2Tr_c[32r+i, f] = 0.5*W2[128c+f, i], replicated to 4 row
                # groups via DMA broadcast of the transposed slice.
                for c in range(HCH):
                    st = mst2p.tile([128, 128], f32, tag="mst2b", name="mst2b")
                    for r in range(NBLK):
                        nc.sync.dma_start(
                            out=st[32 * r:32 * r + OUT, :],
                            in_=W2[128 * c:128 * (c + 1), :].rearrange("m i -> i m"),
                        )
                    nc.vector.tensor_scalar_mul(W2Tr[c], st[:], 0.5)

                # ublk2 = -(c_h@W2) over the s-chunks only (phi streams s
                # there; h'-chunks stream h' so no correction needed).
                ublk2 = mst2p.tile([128, 1024], f32, tag="ublk2", name="ublk2",
                                   bufs=1)
                for half in range(2 * NBLK):
                    j, hf = half // 2, half % 2
                    ps = spsum2.tile([128, 512], f32, tag="spsum2", name="spsum2")
                    for ci, c in enumerate(SC):
                        nc.tensor.matmul(
                            ps[32 * j:32 * j + OUT, :],
                            W2c[c],
                            chv[c][:, 1024 * j + 512 * hf:1024 * j + 512 * (hf + 1)],
                            start=(ci == 0),
                            stop=(ci == len(SC) - 1),
                            tile_position=(0, 32 * j),
                        )
                    nc.scalar.activation(
                        ublk2[32 * j:32 * j + OUT, 512 * hf:512 * (hf + 1)],
                        ps[32 * j:32 * j + OUT, :],
                        Act.Identity,
                        bias=0.0,
                        scale=-2.0,
                    )

                # b2 replicated to rows 32j+i as a per-partition column (x1.0
                # since dbar2 = 2*dbar)
                stb = mst2p.tile([128, 1], f32, tag="b2st", name="b2st", bufs=1)
                nc.vector.memset(stb[:], 0.0)
                for j in range(NBLK):
                    nc.sync.dma_start(out=stb[32 * j:32 * j + OUT, 0:1], in_=b2[:])

                # dbar2 = 0.5*onehot + b2 - (c_h@W2)  (= 2*dbar).  tgt lands
                # on rows {0,32,64,96}; R2 matmul broadcasts to 32-row groups.
                t32 = mst2p.tile([128, 1024], i32, tag="mst3", name="mst3")
                nc.vector.memset(t32[:], 0)
                for j in range(NBLK):
                    nc.sync.dma_start(
                        out=t32[32 * j:32 * j + 1, :],
                        in_=tgt[1024 * j:1024 * (j + 1), 0:1].rearrange("a b -> b a"),
                    )
                tf = mst2p.tile([128, 1024], f32, tag="mst3", name="mst3")
                nc.vector.tensor_copy(tf[:], t32[:])
                tf16 = mst2p.tile([128, 1024], f16, tag="mst3", name="mst3")
                nc.vector.tensor_copy(tf16[:], tf[:])
                eq = mst2p.tile([128, 1024], f32, tag="eqt", name="eqt", bufs=1)
                for hf in range(2):
                    ps = spsum2.tile([128, 512], f32, tag="spsum2", name="spsum2")
                    nc.tensor.matmul(
                        ps[:], R2_t, tf16[:, 512 * hf:512 * (hf + 1)],
                        start=True, stop=True, tile_position=(0, 0),
                    )
                    nc.vector.tensor_scalar(
                        eq[:, 512 * hf:512 * (hf + 1)], ps[:],
                        idxf_t, 0.5, Alu.is_equal, Alu.mult,
                    )
                eq2 = mst2p.tile([128, 1024], f32, tag="eq2", name="eq2", bufs=1)
                nc.vector.tensor_scalar(eq2[:], eq[:], stb, 0.0, Alu.add, Alu.add)
                nc.vector.tensor_tensor(dbar2[:], eq2[:], ublk2[:], Alu.add)

                # y0 blocked
                yst = mst2p.tile([128, 1024], f32, tag="mst3", name="mst3")
                nc.vector.memset(yst[:], 0.0)
                for j in range(NBLK):
                    nc.sync.dma_start(
                        out=yst[32 * j:32 * j + OUT, :],
                        in_=yT[:, 1024 * j:1024 * (j + 1)],
                    )
                ycur = yp.tile([128, 1024], f16, tag="yblk", name="yblk")
                nc.vector.tensor_copy(ycur[:], yst[:])

                # state0: chunks 0,1 -> h'0 = h0; chunks 2,3 -> s0 = h0 + c_h
                s0 = sp.tile([128, HCH * BLOC], f16, tag="s", name="s")
                for c in range(HCH):
                    st = stage2p.tile([128, BLOC], f32, tag="stage2", name="stage2")
                    nc.sync.dma_start(out=st[:], in_=hT[128 * c:128 * (c + 1), :])
                    if c in R1C:
                        nc.vector.tensor_copy(
                            s0[:, BLOC * c:BLOC * (c + 1)], st[:]
                        )
                    else:
                        nc.vector.tensor_tensor(
                            s0[:, BLOC * c:BLOC * (c + 1)], st[:], chv[c][:], Alu.add
                        )
                scur = s0

            # ---------- relaxation loop ----------
            with (
                tc.tile_pool(name="pu", bufs=3, space="PSUM") as pup,
                tc.tile_pool(name="py", bufs=1, space="PSUM") as pyp,
                tc.tile_pool(name="tmp", bufs=4) as tmpp,
                tc.tile_pool(name="hout", bufs=1) as houtp,
            ):
                blocks = [(c, m) for c in range(HCH) for m in CHUNK_MS[c]]
                # DVE-only pair (offloads ACT): chunk 3 blocks (0, 1)
                R4P = {(3, 0), (3, 1)}
                hout = None
                for t in range(T):
                    last = t == T - 1
                    sv = [scur[:, BLOC * c:BLOC * (c + 1)] for c in range(HCH)]
                    if last:
                        hout = houtp.tile(
                            [128, HCH * BLOC], f32, tag="hout", name="hout", bufs=1
                        )
                        snext = None
                    else:
                        snext = sp.tile([128, HCH * BLOC], f16, tag="s", name="s")

                    py = pyp.tile([128, 1024], f32, tag="py", name="py")
                    pend = {}
                    for w, (c, m) in enumerate(blocks):
                        pu = pup.tile([128, 1024], f32, tag="pu", name="pu")
                        mc = slice(1024 * m, 1024 * (m + 1))
                        # identity streams: 0.5*state (+ 0.5*c_h for R1 chunks)
                        for hf in range(2):
                            cs = slice(1024 * m + 512 * hf, 1024 * m + 512 * (hf + 1))
                            ps = slice(512 * hf, 512 * (hf + 1))
                            nc.tensor.matmul(
                                pu[:, ps], halfI_t, sv[c][:, cs],
                                start=True, stop=False, tile_position=(0, 0),
                            )
                            if c in R1C:
                                nc.tensor.matmul(
                                    pu[:, ps], halfI_t, chv[c][:, cs],
                                    start=False, stop=False, tile_position=(0, 0),
                                )
                        # psi: row-strip m only (per-strip LDW; adjacent
                        # blocks have distinct m -> concurrent row groups)
                        nc.tensor.matmul(
                            pu[:, 0:512],
                            W2Tr[c][32 * m:32 * m + OUT, :],
                            ycur[32 * m:32 * m + OUT, 0:512],
                            start=False, stop=False,
                            tile_position=(32 * m, 0),
                        )
                        nc.tensor.matmul(
                            pu[:, 512:1024],
                            W2Tr[c][32 * m:32 * m + OUT, :],
                            ycur[32 * m:32 * m + OUT, 512:1024],
                            start=False, stop=True,
                            tile_position=(32 * m, 0),
                        )

                        # ---- elementwise ----
                        half = w % 2
                        scols = slice(BLOC * c + 1024 * m, BLOC * c + 1024 * (m + 1))
                        if last:
                            # final step: emit h directly (fp32), split
                            # ACT/DVE roughly evenly
                            if (c, m) in R4P or c == 2:
                                nc.vector.tensor_scalar(
                                    hout[:, scols], pu[:],
                                    0.0, 1.0, Alu.max, Alu.min,
                                )
                            else:
                                r = tmpp.tile([128, 1024], f32, tag="lr",
                                              name="lr", bufs=3)
                                nc.scalar.activation(r[:], pu[:], Act.Relu)
                                nc.vector.tensor_scalar(
                                    hout[:, scols], r[:],
                                    1.0, 0.0, Alu.min, Alu.max,
                                )
                            continue

                        if (c, m) in R4P:
                            # DVE-only: clip from PSUM; pair-add c_h
                            if half == 0:
                                rpair = tmpp.tile([128, 2048], f16,
                                                  tag="r4", name="r4", bufs=2)
                                pend[(c, m // 2, "r4")] = rpair
                            else:
                                rpair = pend.pop((c, m // 2, "r4"))
                            nc.vector.tensor_scalar(
                                rpair[:, 1024 * half:1024 * (half + 1)], pu[:],
                                0.0, 1.0, Alu.max, Alu.min,
                            )
                            if half == 1:
                                pc = slice(BLOC * c + 2048 * (m // 2),
                                           BLOC * c + 2048 * (m // 2 + 1))
                                cc = slice(2048 * (m // 2), 2048 * (m // 2 + 1))
                                nc.vector.tensor_tensor(
                                    snext[:, pc], rpair[:], chv[c][:, cc], Alu.add
                                )
                        else:
                            # ACT relu into the pair tile
                            if half == 0:
                                rpair = tmpp.tile([128, 2048], f16,
                                                  tag="rp", name="rp", bufs=3)
                                pend[(c, m // 2, "rp")] = rpair
                            else:
                                rpair = pend.pop((c, m // 2, "rp"))
                            nc.scalar.activation(
                                rpair[:, 1024 * half:1024 * (half + 1)], pu[:],
                                Act.Relu,
                            )
                            if half == 1:
                                pc = slice(BLOC * c + 2048 * (m // 2),
                                           BLOC * c + 2048 * (m // 2 + 1))
                                cc = slice(2048 * (m // 2), 2048 * (m // 2 + 1))
                                if c in R1C:
                                    # state h': just cap at 1 (4x-mode DVE)
                                    nc.vector.tensor_scalar(
                                        snext[:, pc], rpair[:],
                                        1.0, 0.0, Alu.min, Alu.max,
                                    )
                                else:
                                    # state s: min(r,1) + c_h in one stt
                                    nc.vector.scalar_tensor_tensor(
                                        snext[:, pc], rpair[:], 1.0,
                                        chv[c][:, cc], Alu.min, Alu.add,
                                    )

                        # phi for chunk c right after its last block (reads
                        # scur, which stays valid all step)
                        if w % NBLK == NBLK - 1:
                            for hf in range(2):
                                psl = slice(512 * hf, 512 * (hf + 1))
                                for j in range(NBLK):
                                    nc.tensor.matmul(
                                        py[32 * j:32 * j + OUT, psl],
                                        W2c[c],
                                        sv[c][:, 1024 * j + 512 * hf:
                                              1024 * j + 512 * (hf + 1)],
                                        start=(c == 0),
                                        stop=False,
                                        tile_position=(0, 32 * j),
                                    )

                    # ---- y tail: 0.25*y (diag I10 tiles) + dbar2 via halfI ----
                    for j in range(NBLK):
                        for hf in range(2):
                            psl = slice(512 * hf, 512 * (hf + 1))
                            nc.tensor.matmul(
                                py[32 * j:32 * j + OUT, psl],
                                I10q_t[32 * j:32 * j + OUT, :],
                                ycur[32 * j:32 * j + OUT, psl],
                                start=False, stop=False,
                                tile_position=(32 * j, 32 * j),
                            )
                    for hf in range(2):
                        psl = slice(512 * hf, 512 * (hf + 1))
                        nc.tensor.matmul(
                            py[:, psl], halfI_t, dbar2[:, psl],
                            start=False, stop=(hf == 1), tile_position=(0, 0),
                        )
                    ynext = yp.tile([128, 1024], f16, tag="yblk", name="yblk")
                    nc.vector.tensor_scalar(
                        ynext[:], py[:], 0.0, 1.0, Alu.max, Alu.min
                    )

                    if not last:
                        scur = snext
                    ycur = ynext

                # ---------- tail ----------
                for c in range(HCH):
                    nc.sync.dma_start(
                        out=hT_out[128 * c:128 * (c + 1), :],
                        in_=hout[:, BLOC * c:BLOC * (c + 1)],
                    )
                yst2 = tmpp.tile([128, 1024], f32, tag="yo", name="yo", bufs=1)
                nc.vector.tensor_copy(yst2[:], ycur[:])
                for j in range(NBLK):
                    nc.sync.dma_start(
                        out=yT_out[:, 1024 * j:1024 * (j + 1)],
                        in_=yst2[32 * j:32 * j + OUT, :],
                    )

    if not nc.is_finalized():
        nc.finalize()
    return nc


def _consts():
    cst16 = np.zeros((128, CF16_W), dtype=np.float16)
    cst16[:, C_HALFI:C_HALFI + 128] = 0.5 * np.eye(128, dtype=np.float16)
    cst32 = np.zeros((128, 8), dtype=np.float32)
    cst32[:, 0] = -1.0
    for j in range(NBLK):
        for i in range(OUT):
            cst16[32 * j + i, C_I10Q + i] = 0.25
            cst16[32 * j, C_R2 + 32 * j + i] = 1.0
            cst32[32 * j + i, 0] = float(i)
    return cst16, cst32


def prepare(inputs):
    x = np.asarray(inputs["x"], dtype=np.float32)
    h0 = np.asarray(inputs["h_init"], dtype=np.float32)
    y0 = np.asarray(inputs["y_init"], dtype=np.float32)
    W1 = np.ascontiguousarray(np.asarray(inputs["W1"], dtype=np.float32))
    W2 = np.ascontiguousarray(np.asarray(inputs["W2"], dtype=np.float32))
    b1 = np.ascontiguousarray(
        np.asarray(inputs["b1"], dtype=np.float32).reshape(HID, 1)
    )
    b2 = np.ascontiguousarray(
        np.asarray(inputs["b2"], dtype=np.float32).reshape(OUT, 1)
    )
    target = np.ascontiguousarray(inputs["target"])
    T = int(inputs["T"])

    xT = np.ascontiguousarray(x.T)      # [IN, B]
    hT = np.ascontiguousarray(h0.T)     # [HID, B]
    yT = np.ascontiguousarray(y0.T)     # [OUT, B]
    if target.dtype == np.int64:
        tgt32 = target.view(np.int32).reshape(B, 2)  # int64 -> (lo, hi) pairs
    else:
        tgt32 = np.zeros((B, 2), dtype=np.int32)
        tgt32[:, 0] = target

    key = T
    if key not in _BUILT:
        _BUILT[key] = _build(T)
    nc = _BUILT[key]

    cst16, cst32 = _consts()
    in_maps = []
    for k in range(N_CORES):
        sl = slice(k * BLOC, (k + 1) * BLOC)
        in_maps.append({
            "xT": np.ascontiguousarray(xT[:, sl]),
            "hT": np.ascontiguousarray(hT[:, sl]),
            "yT": np.ascontiguousarray(yT[:, sl]),
            "W1": W1, "W2": W2, "b1": b1, "b2": b2,
            "tgt": np.ascontiguousarray(tgt32[sl]),
            "cst16": cst16, "cst32": cst32,
        })

    return nc, in_maps


def assemble(results):
    out = np.empty((B, HID + OUT), dtype=np.float32)
    for k in range(N_CORES):
        sl = slice(k * BLOC, (k + 1) * BLOC)
        out[sl, :HID] = np.asarray(results[k]["hT_out"]).T
        out[sl, HID:] = np.asarray(results[k]["yT_out"]).T
    return out


def kernel(**inputs):
    from concourse import bass_utils

    nc, in_maps = prepare(inputs)
    res = bass_utils.run_bass_kernel_spmd(nc, in_maps, list(range(N_CORES)))
    globals()["_LAST_RESULTS"] = res
    return assemble(res.results)


# revision 15
# speedup vs baseline: 1.3403x; 1.3403x over previous
# Trainium2 Bass kernel for nn_EqPropNetwork (equilibrium-propagation relaxation).
#
# Math (per reference.py):
#   c_h = x @ W1 + b1                                  [B, HID]  (constant over steps)
#   repeat T times:
#     psi = y @ W2.T ; phi = h @ W2
#     h'  = clip(0.5*h + 0.5*c_h + 0.5*psi, 0, 1)
#     y'  = clip(0.25*y + 0.5*phi + 0.5*b2 + 0.25*onehot(target), 0, 1)
#   out = concat(h, y)                                  [B, HID+OUT]
#
# Mapping (per core, B_loc = 4096, pure data parallel over 8 cores):
#   Feature-major state: partition = feature, free = batch; 4 chunks x 4
#   batch-blocks of [128, 1024].  Engine-balanced routes per chunk:
#   * chunks 0,1 ("R1"): state = h' (post-clip).  PE identity streams BOTH
#     h' and c_h (PSUM u = 0.5h' + 0.5c_h + 0.5psi); ACT relu evacuates
#     PSUM; DVE finishes with a 4x-mode min/max.  (PE-side add is cheaper
#     than a DVE tensor_tensor add.)
#   * chunks 2,3: state = s := h + c_h.  PSUM u = 0.5s + 0.5psi.  Most
#     blocks: ACT relu + one DVE scalar_tensor_tensor (min,add); one pair
#     runs DVE-only (clip from PSUM + 2x tensor_tensor add) to offload ACT.
#   * y-update rides PE: py = 0.5*(s@W2) + 0.25*y (diag-tiled I10 matmuls)
#     + dbar (identity matmul streaming dbar2 = 2*dbar), then a single DVE
#     clip.  dbar folds onehot/4 + b2/2 - 0.5*(c_h@W2) over the s-chunks.
#   * psi matmuls: per-strip LDWEIGHTS (rows 32m..32m+10 only) and block
#     order rotates the batch-block m so adjacent psi matmuls land on
#     distinct PE row groups and overlap.  phi matmuls are 4-way
#     col-group packed.
import sys

import numpy as np

if "/opt/trn_rl_repo" not in sys.path:
    sys.path.insert(0, "/opt/trn_rl_repo")

N_CORES = 8
B, IN, HID, OUT = 32768, 784, 512, 10
BLOC = B // N_CORES  # 4096
NBLK = BLOC // 1024  # 4 batch blocks of 1024
KIN = 7              # IN chunks of 112
KC = IN // KIN       # 112
HCH = HID // 128     # 4 hidden chunks

R1C = (0, 1)         # chunks with h'-state (PE adds c_h)
SC = (2, 3)          # chunks with s-state

# packed fp16 const tile column offsets
C_HALFI = 0          # [128, 128] 0.5*I128
C_W2TR = 128         # 4 x [128, 128] W2T replicated, pre-scaled 0.5
C_I10Q = 640         # [128, 10] 0.25*I10 at 4 row offsets
C_R2 = 650           # [128, 128] rep: R2[32j, 32j+i]=1 (i<10) -> bcast block rows
CF16_W = 778

# per-chunk block (batch-block m) orders: chosen so waves of 3 consecutive
# blocks get distinct m (psi row-group overlap) while pair halves stay in
# the same 2048-col group (m//2) for paired DVE ops
CHUNK_MS = {0: (0, 1, 2, 3), 1: (1, 0, 3, 2), 2: (1, 0, 3, 2), 3: (3, 2, 0, 1)}

_BUILT = {}


def _build(T):
    import concourse.bass as bass
    from concourse import bacc, mybir
    from concourse.tile import TileContext

    f32 = mybir.dt.float32
    f16 = mybir.dt.float16
    i32 = mybir.dt.int32
    Alu = mybir.AluOpType
    Act = mybir.ActivationFunctionType

    nc = bacc.Bacc("TRN2", target_bir_lowering=False)

    xT = nc.declare_dram_parameter("xT", [IN, BLOC], f32, isOutput=False)
    hT = nc.declare_dram_parameter("hT", [HID, BLOC], f32, isOutput=False)
    yT = nc.declare_dram_parameter("yT", [OUT, BLOC], f32, isOutput=False)
    W1 = nc.declare_dram_parameter("W1", [IN, HID], f32, isOutput=False)
    W2 = nc.declare_dram_parameter("W2", [HID, OUT], f32, isOutput=False)
    b1 = nc.declare_dram_parameter("b1", [HID, 1], f32, isOutput=False)
    b2 = nc.declare_dram_parameter("b2", [OUT, 1], f32, isOutput=False)
    tgt = nc.declare_dram_parameter("tgt", [BLOC, 2], i32, isOutput=False)
    cst16 = nc.declare_dram_parameter("cst16", [128, CF16_W], f16, isOutput=False)
    cst32 = nc.declare_dram_parameter("cst32", [128, 8], f32, isOutput=False)

    hT_out = nc.declare_dram_parameter("hT_out", [HID, BLOC], f32, isOutput=True)
    yT_out = nc.declare_dram_parameter("yT_out", [OUT, BLOC], f32, isOutput=True)

    with TileContext(nc) as tc:
        with (
            tc.tile_pool(name="const", bufs=1) as constp,
            tc.tile_pool(name="ch", bufs=1) as chp,
            tc.tile_pool(name="state", bufs=2) as sp,
            tc.tile_pool(name="ypool", bufs=2) as yp,
        ):
            cf16 = constp.tile([128, CF16_W], f16, tag="cf16", name="cf16")
            cf32 = constp.tile([128, 8], f32, tag="cf32", name="cf32")
            dbar2 = constp.tile([128, 1024], f16, tag="dbar2", name="dbar2")
            cb1 = constp.tile([128, HCH], f32, tag="cb1", name="cb1")
            nc.sync.dma_start(out=cf16[:], in_=cst16[:])
            nc.sync.dma_start(out=cf32[:], in_=cst32[:])
            nc.sync.dma_start(
                out=cb1.rearrange("p (c o) -> p c o", c=HCH),
                in_=b1.rearrange("(c p) o -> p c o", c=HCH),
            )

            halfI_t = cf16[:, C_HALFI:C_HALFI + 128]
            W2Tr = [cf16[:, C_W2TR + 128 * c:C_W2TR + 128 * (c + 1)]
                    for c in range(HCH)]
            I10q_t = cf16[:, C_I10Q:C_I10Q + OUT]
            idxf_t = cf32[:, 0:1]
            b1c = [cb1[:, c:c + 1] for c in range(HCH)]
            R2_t = cf16[:, C_R2:C_R2 + 128]

            ch = chp.tile([128, HCH * BLOC], f16, tag="ch", name="ch")
            chv = [ch[:, BLOC * c:BLOC * (c + 1)] for c in range(HCH)]

            # ---------- setup phase A: c_h = x@W1 + b1 ----------
            # x loaded in batch stripes of 512: a stripe's x@W1 matmuls start
            # as soon as its 7 IN-chunks land (DMA overlaps PE).
            with (
                tc.tile_pool(name="x16p", bufs=1) as x16p,
                tc.tile_pool(name="stage", bufs=8) as stagep,
                tc.tile_pool(name="mst", bufs=3) as mstp,
                tc.tile_pool(name="spsum", bufs=4, space="PSUM") as spsum,
            ):
                w1_16 = x16p.tile([128, KIN * HID], f16, tag="w1_16", name="w1_16")
                for k in range(KIN):
                    st = mstp.tile([128, HID], f32, tag="mst", name="mst")
                    nc.sync.dma_start(out=st[:KC, :], in_=W1[KC * k:KC * (k + 1), :])
                    nc.vector.tensor_copy(
                        w1_16[:KC, HID * k:HID * (k + 1)], st[:KC, :]
                    )
                x16 = x16p.tile([128, KIN * BLOC], f16, tag="x16", name="x16")
                for blk in range(BLOC // 512):
                    bsl = slice(512 * blk, 512 * (blk + 1))
                    for k in range(KIN):
                        st = stagep.tile([128, 512], f32, tag="stage", name="stage")
                        nc.sync.dma_start(
                            out=st[:KC, :], in_=xT[KC * k:KC * (k + 1), bsl]
                        )
                        nc.vector.tensor_copy(
                            x16[:KC, BLOC * k + 512 * blk:BLOC * k + 512 * (blk + 1)],
                            st[:KC, :],
                        )
                    for c in range(HCH):
                        ps = spsum.tile([128, 512], f32, tag="spsum", name="spsum")
                        for k in range(KIN):
                            nc.tensor.matmul(
                                ps[:],
                                w1_16[:KC, HID * k + 128 * c:HID * k + 128 * (c + 1)],
                                x16[:KC, BLOC * k + 512 * blk:BLOC * k + 512 * (blk + 1)],
                                start=(k == 0),
                                stop=(k == KIN - 1),
                                tile_position=(0, 0),
                            )
                        nc.scalar.activation(
                            chv[c][:, 512 * blk:512 * (blk + 1)],
                            ps[:],
                            Act.Identity,
                            bias=b1c[c],
                            scale=1.0,
                        )

            # ---------- setup phase B: W2 forms, dbar2, y0, state0 ----------
            with (
                tc.tile_pool(name="stage2", bufs=2) as stage2p,
                tc.tile_pool(name="mst2", bufs=3) as mst2p,
                tc.tile_pool(name="spsum2", bufs=4, space="PSUM") as spsum2,
            ):
                # W2c = 0.5*W2 chunks [128, 4*10] (phi stationaries);
                # lives in the persistent const pool (used all loop)
                w2c16 = constp.tile([128, HCH * OUT], f16, tag="w2c16",
                                    name="w2c16")
                st = mst2p.tile([128, HCH * OUT], f32, tag="mst2", name="mst2")
                nc.sync.dma_start(
                    out=st.rearrange("p (c i) -> p c i", c=HCH),
                    in_=W2.rearrange("(c p) i -> p c i", c=HCH),
                )
                nc.vector.tensor_scalar_mul(w2c16[:], st[:], 0.5)
                W2c = [w2c16[:, OUT * c:OUT * (c + 1)] for c in range(HCH)]

                # W2Tr_c[32r+i, f] = 0.5*W2[128c+f, i], replicated to 4 row
                # groups via DMA broadcast of the transposed slice.
                for c in range(HCH):
                    st = mst2p.tile([128, 128], f32, tag="mst2b", name="mst2b")
                    for r in range(NBLK):
                        nc.sync.dma_start(
                            out=st[32 * r:32 * r + OUT, :],
                            in_=W2[128 * c:128 * (c + 1), :].rearrange("m i -> i m"),
                        )
                    nc.vector.tensor_scalar_mul(W2Tr[c], st[:], 0.5)

                # ublk2 = -(c_h@W2) over the s-chunks only (phi streams s
                # there; h'-chunks stream h' so no correction needed).
                ublk2 = mst2p.tile([128, 1024], f32, tag="ublk2", name="ublk2",
                                   bufs=1)
                for half in range(2 * NBLK):
                    j, hf = half // 2, half % 2
                    ps = spsum2.tile([128, 512], f32, tag="spsum2", name="spsum2")
                    for ci, c in enumerate(SC):
                        nc.tensor.matmul(
                            ps[32 * j:32 * j + OUT, :],
                            W2c[c],
                            chv[c][:, 1024 * j + 512 * hf:1024 * j + 512 * (hf + 1)],
                            start=(ci == 0),
                            stop=(ci == len(SC) - 1),
                            tile_position=(0, 32 * j),
                        )
                    nc.scalar.activation(
                        ublk2[32 * j:32 * j + OUT, 512 * hf:512 * (hf + 1)],
                        ps[32 * j:32 * j + OUT, :],
                        Act.Identity,
                        bias=0.0,
                        scale=-2.0,
                    )

                # b2 replicated to rows 32j+i as a per-partition column (x1.0
                # since dbar2 = 2*dbar)
                stb = mst2p.tile([128, 1], f32, tag="b2st", name="b2st", bufs=1)
                nc.vector.memset(stb[:], 0.0)
                for j in range(NBLK):
                    nc.sync.dma_start(out=stb[32 * j:32 * j + OUT, 0:1], in_=b2[:])

                # dbar2 = 0.5*onehot + b2 - (c_h@W2)  (= 2*dbar).  tgt lands
                # on rows {0,32,64,96}; R2 matmul broadcasts to 32-row groups.
                t32 = mst2p.tile([128, 1024], i32, tag="mst3", name="mst3")
                nc.vector.memset(t32[:], 0)
                for j in range(NBLK):
                    nc.sync.dma_start(
                        out=t32[32 * j:32 * j + 1, :],
                        in_=tgt[1024 * j:1024 * (j + 1), 0:1].rearrange("a b -> b a"),
                    )
                tf = mst2p.tile([128, 1024], f32, tag="mst3", name="mst3")
                nc.vector.tensor_copy(tf[:], t32[:])
                tf16 = mst2p.tile([128, 1024], f16, tag="mst3", name="mst3")
                nc.vector.tensor_copy(tf16[:], tf[:])
                eq = mst2p.tile([128, 1024], f32, tag="eqt", name="eqt", bufs=1)
                for hf in range(2):
                    ps = spsum2.tile([128, 512], f32, tag="spsum2", name="spsum2")
                    nc.tensor.matmul(
                        ps[:], R2_t, tf16[:, 512 * hf:512 * (hf + 1)],
                        start=True, stop=True, tile_position=(0, 0),
                    )
                    nc.vector.tensor_scalar(
                        eq[:, 512 * hf:512 * (hf + 1)], ps[:],
                        idxf_t, 0.5, Alu.is_equal, Alu.mult,
                    )
                eq2 = mst2p.tile([128, 1024], f32, tag="eq2", name="eq2", bufs=1)
                nc.vector.tensor_scalar(eq2[:], eq[:], stb, 0.0, Alu.add, Alu.add)
                nc.vector.tensor_tensor(dbar2[:], eq2[:], ublk2[:], Alu.add)

                # y0 blocked
                yst = mst2p.tile([128, 1024], f32, tag="mst3", name="mst3")
                nc.vector.memset(yst[:], 0.0)
                for j in range(NBLK):
                    nc.sync.dma_start(
                        out=yst[32 * j:32 * j + OUT, :],
                        in_=yT[:, 1024 * j:1024 * (j + 1)],
                    )
                ycur = yp.tile([128, 1024], f16, tag="yblk", name="yblk")
                nc.vector.tensor_copy(ycur[:], yst[:])

                # state0: chunks 0,1 -> h'0 = h0; chunks 2,3 -> s0 = h0 + c_h
                s0 = sp.tile([128, HCH * BLOC], f16, tag="s", name="s")
                for c in range(HCH):
                    st = stage2p.tile([128, BLOC], f32, tag="stage2", name="stage2")
                    nc.sync.dma_start(out=st[:], in_=hT[128 * c:128 * (c + 1), :])
                    if c in R1C:
                        nc.vector.tensor_copy(
                            s0[:, BLOC * c:BLOC * (c + 1)], st[:]
                        )
                    else:
                        nc.vector.tensor_tensor(
                            s0[:, BLOC * c:BLOC * (c + 1)], st[:], chv[c][:], Alu.add
                        )
                scur = s0

            # ---------- relaxation loop ----------
            with (
                tc.tile_pool(name="pu", bufs=3, space="PSUM") as pup,
                tc.tile_pool(name="py", bufs=1, space="PSUM") as pyp,
                tc.tile_pool(name="tmp", bufs=4) as tmpp,
                tc.tile_pool(name="hout", bufs=1) as houtp,
            ):
                blocks = [(c, m) for c in range(HCH) for m in CHUNK_MS[c]]
                # DVE-only pairs (offload ACT): two pairs in the s-chunks
                R4P = {(2, 0), (2, 1), (3, 0), (3, 1)}
                for t in range(T):
                    last = t == T - 1
                    sv = [scur[:, BLOC * c:BLOC * (c + 1)] for c in range(HCH)]
                    hc = None
                    if last:
                        snext = None
                    else:
                        snext = sp.tile([128, HCH * BLOC], f16, tag="s", name="s")

                    py = pyp.tile([128, 1024], f32, tag="py", name="py")
                    pend = {}
                    waves = [blocks[i:i + 3] for i in range(0, len(blocks), 3)]
                    wbase = 0
                    for wave in waves:
                      pus = []
                      # identity streams for the whole wave, back-to-back
                      for c, m in wave:
                        pu = pup.tile([128, 1024], f32, tag="pu", name="pu")
                        pus.append(pu)
                        for hf in range(2):
                            cs = slice(1024 * m + 512 * hf, 1024 * m + 512 * (hf + 1))
                            ps = slice(512 * hf, 512 * (hf + 1))
                            nc.tensor.matmul(
                                pu[:, ps], halfI_t, sv[c][:, cs],
                                start=True, stop=False, tile_position=(0, 0),
                            )
                            if c in R1C:
                                nc.tensor.matmul(
                                    pu[:, ps], halfI_t, chv[c][:, cs],
                                    start=False, stop=False, tile_position=(0, 0),
                                )
                      # psi for the wave: adjacent matmuls on distinct row
                      # groups (per-strip LDW) -> concurrent
                      for (c, m), pu in zip(wave, pus):
                        nc.tensor.matmul(
                            pu[:, 0:512],
                            W2Tr[c][32 * m:32 * m + OUT, :],
                            ycur[32 * m:32 * m + OUT, 0:512],
                            start=False, stop=False,
                            tile_position=(32 * m, 0),
                        )
                      for (c, m), pu in zip(wave, pus):
                        nc.tensor.matmul(
                            pu[:, 512:1024],
                            W2Tr[c][32 * m:32 * m + OUT, :],
                            ycur[32 * m:32 * m + OUT, 512:1024],
                            start=False, stop=True,
                            tile_position=(32 * m, 0),
                        )
                      for wi, ((c, m), pu) in enumerate(zip(wave, pus)):
                        w = wbase + wi
                        # phi for chunk c right after its last block (reads
                        # scur, which stays valid all step)
                        if w % NBLK == NBLK - 1:
                            for hf in range(2):
                                psl = slice(512 * hf, 512 * (hf + 1))
                                for j in range(NBLK):
                                    nc.tensor.matmul(
                                        py[32 * j:32 * j + OUT, psl],
                                        W2c[c],
                                        sv[c][:, 1024 * j + 512 * hf:
                                              1024 * j + 512 * (hf + 1)],
                                        start=(c == 0),
                                        stop=False,
                                        tile_position=(0, 32 * j),
                                    )
                        # ---- elementwise ----
                        half = m % 2
                        mcols = slice(1024 * m, 1024 * (m + 1))
                        if last:
                            # final step: emit h (fp32) per chunk, DMA as
                            # each chunk completes; split ACT/DVE evenly
                            if w % NBLK == 0:
                                hc = houtp.tile([128, BLOC], f32, tag="hout",
                                                name="hout", bufs=2)
                            if (c, m) in R4P or c == 2:
                                nc.vector.tensor_scalar(
                                    hc[:, mcols], pu[:],
                                    0.0, 1.0, Alu.max, Alu.min,
                                )
                            else:
                                r = tmpp.tile([128, 1024], f32, tag="lr",
                                              name="lr", bufs=2)
                                nc.scalar.activation(r[:], pu[:], Act.Relu)
                                nc.vector.tensor_scalar(
                                    hc[:, mcols], r[:],
                                    1.0, 0.0, Alu.min, Alu.max,
                                )
                            if w % NBLK == NBLK - 1:
                                nc.sync.dma_start(
                                    out=hT_out[128 * c:128 * (c + 1), :],
                                    in_=hc[:],
                                )
                            continue

                        if (c, m) in R4P:
                            # DVE-only: clip from PSUM; pair-add c_h
                            key = (c, m // 2, "r4")
                            done = key in pend
                            if done:
                                rpair = pend.pop(key)
                            else:
                                rpair = tmpp.tile([128, 2048], f16,
                                                  tag="r4", name="r4", bufs=2)
                                pend[key] = rpair
                            nc.vector.tensor_scalar(
                                rpair[:, 1024 * half:1024 * (half + 1)], pu[:],
                                0.0, 1.0, Alu.max, Alu.min,
                            )
                            if done:
                                pc = slice(BLOC * c + 2048 * (m // 2),
                                           BLOC * c + 2048 * (m // 2 + 1))
                                cc = slice(2048 * (m // 2), 2048 * (m // 2 + 1))
                                nc.vector.tensor_tensor(
                                    snext[:, pc], rpair[:], chv[c][:, cc], Alu.add
                                )
                        else:
                            # ACT relu into the pair tile
                            key = (c, m // 2, "rp")
                            done = key in pend
                            if done:
                                rpair = pend.pop(key)
                            else:
                                rpair = tmpp.tile([128, 2048], f16,
                                                  tag="rp", name="rp", bufs=3)
                                pend[key] = rpair
                            nc.scalar.activation(
                                rpair[:, 1024 * half:1024 * (half + 1)], pu[:],
                                Act.Relu,
                            )
                            if done:
                                pc = slice(BLOC * c + 2048 * (m // 2),
                                           BLOC * c + 2048 * (m // 2 + 1))
                                cc = slice(2048 * (m // 2), 2048 * (m // 2 + 1))
                                if c in R1C:
                                    # state h': just cap at 1 (4x-mode DVE)
                                    nc.vector.tensor_scalar(
                                        snext[:, pc], rpair[:],
                                        1.0, 0.0, Alu.min, Alu.max,
                                    )
                                else:
                                    # state s: 4x-mode min then 2x-mode add
                                    # (two fast ops beat one 1x-mode stt)
                                    rm = tmpp.tile([128, 2048], f16,
                                                   tag="rm", name="rm", bufs=2)
                                    nc.vector.tensor_scalar(
                                        rm[:], rpair[:],
                                        1.0, 0.0, Alu.min, Alu.max,
                                    )
                                    nc.vector.tensor_tensor(
                                        snext[:, pc], rm[:], chv[c][:, cc],
                                        Alu.add,
                                    )

                      wbase += len(wave)

                    # ---- y tail: 0.25*y (diag I10 tiles) + dbar2 via halfI ----
                    for j in range(NBLK):
                        for hf in range(2):
                            psl = slice(512 * hf, 512 * (hf + 1))
                            nc.tensor.matmul(
                                py[32 * j:32 * j + OUT, psl],
                                I10q_t[32 * j:32 * j + OUT, :],
                                ycur[32 * j:32 * j + OUT, psl],
                                start=False, stop=False,
                                tile_position=(32 * j, 32 * j),
                            )
                    for hf in range(2):
                        psl = slice(512 * hf, 512 * (hf + 1))
                        nc.tensor.matmul(
                            py[:, psl], halfI_t, dbar2[:, psl],
                            start=False, stop=(hf == 1), tile_position=(0, 0),
                        )
                    ynext = yp.tile([128, 1024], f16, tag="yblk", name="yblk")
                    nc.vector.tensor_scalar(
                        ynext[:], py[:], 0.0, 1.0, Alu.max, Alu.min
                    )

                    if not last:
                        scur = snext
                    ycur = ynext

                # ---------- tail ----------
                yst2 = tmpp.tile([128, 1024], f32, tag="yo", name="yo", bufs=1)
                nc.vector.tensor_copy(yst2[:], ycur[:])
                for j in range(NBLK):
                    nc.sync.dma_start(
                        out=yT_out[:, 1024 * j:1024 * (j + 1)],
                        in_=yst2[32 * j:32 * j + OUT, :],
                    )

    if not nc.is_finalized():
        nc.finalize()
    return nc


def _consts():
    cst16 = np.zeros((128, CF16_W), dtype=np.float16)
    cst16[:, C_HALFI:C_HALFI + 128] = 0.5 * np.eye(128, dtype=np.float16)
    cst32 = np.zeros((128, 8), dtype=np.float32)
    cst32[:, 0] = -1.0
    for j in range(NBLK):
        for i in range(OUT):
            cst16[32 * j + i, C_I10Q + i] = 0.25
            cst16[32 * j, C_R2 + 32 * j + i] = 1.0
            cst32[32 * j + i, 0] = float(i)
    return cst16, cst32


def prepare(inputs):
    x = np.asarray(inputs["x"], dtype=np.float32)
    h0 = np.asarray(inputs["h_init"], dtype=np.float32)
    y0 = np.asarray(inputs["y_init"], dtype=np.float32)
    W1 = np.ascontiguousarray(np.asarray(inputs["W1"], dtype=np.float32))
    W2 = np.ascontiguousarray(np.asarray(inputs["W2"], dtype=np.float32))
    b1 = np.ascontiguousarray(
        np.asarray(inputs["b1"], dtype=np.float32).reshape(HID, 1)
    )
    b2 = np.ascontiguousarray(
        np.asarray(inputs["b2"], dtype=np.float32).reshape(OUT, 1)
    )
    target = np.ascontiguousarray(inputs["target"])
    T = int(inputs["T"])

    xT = np.ascontiguousarray(x.T)      # [IN, B]
    hT = np.ascontiguousarray(h0.T)     # [HID, B]
    yT = np.ascontiguousarray(y0.T)     # [OUT, B]
    if target.dtype == np.int64:
        tgt32 = target.view(np.int32).reshape(B, 2)  # int64 -> (lo, hi) pairs
    else:
        tgt32 = np.zeros((B, 2), dtype=np.int32)
        tgt32[:, 0] = target

    key = T
    if key not in _BUILT:
        _BUILT[key] = _build(T)
    nc = _BUILT[key]

    cst16, cst32 = _consts()
    in_maps = []
    for k in range(N_CORES):
        sl = slice(k * BLOC, (k + 1) * BLOC)
        in_maps.append({
            "xT": np.ascontiguousarray(xT[:, sl]),
            "hT": np.ascontiguousarray(hT[:, sl]),
            "yT": np.ascontiguousarray(yT[:, sl]),
            "W1": W1, "W2": W2, "b1": b1, "b2": b2,
            "tgt": np.ascontiguousarray(tgt32[sl]),
            "cst16": cst16, "cst32": cst32,
        })

    return nc, in_maps


def assemble(results):
    out = np.empty((B, HID + OUT), dtype=np.float32)
    for k in range(N_CORES):
        sl = slice(k * BLOC, (k + 1) * BLOC)
        out[sl, :HID] = np.asarray(results[k]["hT_out"]).T
        out[sl, HID:] = np.asarray(results[k]["yT_out"]).T
    return out


def kernel(**inputs):
    from concourse import bass_utils

    nc, in_maps = prepare(inputs)
    res = bass_utils.run_bass_kernel_spmd(nc, in_maps, list(range(N_CORES)))
    globals()["_LAST_RESULTS"] = res
    return assemble(res.results)


# revision 24
# speedup vs baseline: 1.4703x; 1.0969x over previous
# Trainium2 Bass kernel for nn_EqPropNetwork (equilibrium-propagation relaxation).
#
# Math (per reference.py):
#   c_h = x @ W1 + b1                                  [B, HID]  (constant over steps)
#   repeat T times:
#     psi = y @ W2.T ; phi = h @ W2
#     h'  = clip(0.5*h + 0.5*c_h + 0.5*psi, 0, 1)
#     y'  = clip(0.25*y + 0.5*phi + 0.5*b2 + 0.25*onehot(target), 0, 1)
#   out = concat(h, y)                                  [B, HID+OUT]
#
# Mapping (per core, B_loc = 4096, pure data parallel over 8 cores):
#   Feature-major state: partition = feature, free = batch; 4 chunks x 4
#   batch-blocks of [128, 1024].  Engine-balanced routes per chunk:
#   * chunks 0,1 ("R1"): state = h' (post-clip).  PE identity streams BOTH
#     h' and c_h (PSUM u = 0.5h' + 0.5c_h + 0.5psi); ACT relu evacuates
#     PSUM; DVE finishes with a 4x-mode min/max.  (PE-side add is cheaper
#     than a DVE tensor_tensor add.)
#   * chunks 2,3: state = s := h + c_h.  PSUM u = 0.5s + 0.5psi.  Most
#     blocks: ACT relu + one DVE scalar_tensor_tensor (min,add); one pair
#     runs DVE-only (clip from PSUM + 2x tensor_tensor add) to offload ACT.
#   * y-update rides PE: py = 0.5*(s@W2) + 0.25*y (diag-tiled I10 matmuls)
#     + dbar (identity matmul streaming dbar2 = 2*dbar), then a single DVE
#     clip.  dbar folds onehot/4 + b2/2 - 0.5*(c_h@W2) over the s-chunks.
#   * psi matmuls: per-strip LDWEIGHTS (rows 32m..32m+10 only) and block
#     order rotates the batch-block m so adjacent psi matmuls land on
#     distinct PE row groups and overlap.  phi matmuls are 4-way
#     col-group packed.
import sys

import numpy as np

if "/opt/trn_rl_repo" not in sys.path:
    sys.path.insert(0, "/opt/trn_rl_repo")

N_CORES = 8
B, IN, HID, OUT = 32768, 784, 512, 10
BLOC = B // N_CORES  # 4096
NBLK = BLOC // 1024  # 4 batch blocks of 1024
KIN = 7              # IN chunks of 112
KC = IN // KIN       # 112
HCH = HID // 128     # 4 hidden chunks

R1C = (0, 1)         # chunks with h'-state (PE adds c_h)
SC = (2, 3)          # chunks with s-state

# packed fp16 const tile column offsets
C_HALFI = 0          # [128, 128] 0.5*I128
C_W2TR = 128         # 4 x [128, 128] W2T replicated, pre-scaled 0.5
C_I10Q = 640         # [128, 10] 0.25*I10 at 4 row offsets
C_R2 = 650           # [128, 128] rep: R2[32j, 32j+i]=1 (i<10) -> bcast block rows
CF16_W = 778

# per-chunk block (batch-block m) orders: chosen so waves of 3 consecutive
# blocks get distinct m (psi row-group overlap) while pair halves stay in
# the same 2048-col group (m//2) for paired DVE ops
CHUNK_MS = {0: (0, 1, 2, 3), 1: (1, 0, 3, 2), 2: (1, 0, 3, 2), 3: (3, 2, 0, 1)}

_BUILT = {}


def _build(T):
    import concourse.bass as bass
    from concourse import bacc, mybir
    from concourse.tile import TileContext

    f32 = mybir.dt.float32
    f16 = mybir.dt.float16
    i32 = mybir.dt.int32
    Alu = mybir.AluOpType
    Act = mybir.ActivationFunctionType

    nc = bacc.Bacc("TRN2", target_bir_lowering=False)

    xT = nc.declare_dram_parameter("xT", [IN, BLOC], f32, isOutput=False)
    hT = nc.declare_dram_parameter("hT", [HID, BLOC], f32, isOutput=False)
    yT = nc.declare_dram_parameter("yT", [OUT, BLOC], f32, isOutput=False)
    W1 = nc.declare_dram_parameter("W1", [IN, HID], f32, isOutput=False)
    W2 = nc.declare_dram_parameter("W2", [HID, OUT], f32, isOutput=False)
    b1 = nc.declare_dram_parameter("b1", [HID, 1], f32, isOutput=False)
    b2 = nc.declare_dram_parameter("b2", [OUT, 1], f32, isOutput=False)
    tgt = nc.declare_dram_parameter("tgt", [BLOC, 2], i32, isOutput=False)
    cst16 = nc.declare_dram_parameter("cst16", [128, CF16_W], f16, isOutput=False)
    cst32 = nc.declare_dram_parameter("cst32", [128, 8], f32, isOutput=False)

    hT_out = nc.declare_dram_parameter("hT_out", [HID, BLOC], f32, isOutput=True)
    yT_out = nc.declare_dram_parameter("yT_out", [OUT, BLOC], f32, isOutput=True)

    with TileContext(nc) as tc:
        with (
            tc.tile_pool(name="const", bufs=1) as constp,
            tc.tile_pool(name="ch", bufs=1) as chp,
            tc.tile_pool(name="state", bufs=2) as sp,
            tc.tile_pool(name="ypool", bufs=2) as yp,
        ):
            cf16 = constp.tile([128, CF16_W], f16, tag="cf16", name="cf16")
            cf32 = constp.tile([128, 8], f32, tag="cf32", name="cf32")
            dbar2 = constp.tile([128, 1024], f16, tag="dbar2", name="dbar2")
            cb1 = constp.tile([128, HCH], f32, tag="cb1", name="cb1")
            nc.sync.dma_start(out=cf16[:], in_=cst16[:])
            nc.sync.dma_start(out=cf32[:], in_=cst32[:])
            nc.sync.dma_start(
                out=cb1.rearrange("p (c o) -> p c o", c=HCH),
                in_=b1.rearrange("(c p) o -> p c o", c=HCH),
            )

            halfI_t = cf16[:, C_HALFI:C_HALFI + 128]
            W2Tr = [cf16[:, C_W2TR + 128 * c:C_W2TR + 128 * (c + 1)]
                    for c in range(HCH)]
            I10q_t = cf16[:, C_I10Q:C_I10Q + OUT]
            idxf_t = cf32[:, 0:1]
            b1c = [cb1[:, c:c + 1] for c in range(HCH)]
            R2_t = cf16[:, C_R2:C_R2 + 128]

            ch = chp.tile([128, HCH * BLOC], f16, tag="ch", name="ch")
            chv = [ch[:, BLOC * c:BLOC * (c + 1)] for c in range(HCH)]

            # ---------- setup phase A: c_h = x@W1 + b1 ----------
            # x loaded in batch stripes of 512: a stripe's x@W1 matmuls start
            # as soon as its 7 IN-chunks land (DMA overlaps PE).
            with (
                tc.tile_pool(name="x16p", bufs=1) as x16p,
                tc.tile_pool(name="stage", bufs=8) as stagep,
                tc.tile_pool(name="mst", bufs=3) as mstp,
                tc.tile_pool(name="spsum", bufs=4, space="PSUM") as spsum,
            ):
                w1_16 = x16p.tile([128, KIN * HID], f16, tag="w1_16", name="w1_16")
                for k in range(KIN):
                    st = mstp.tile([128, HID], f32, tag="mst", name="mst")
                    nc.sync.dma_start(out=st[:KC, :], in_=W1[KC * k:KC * (k + 1), :])
                    nc.vector.tensor_copy(
                        w1_16[:KC, HID * k:HID * (k + 1)], st[:KC, :]
                    )
                x16 = x16p.tile([128, KIN * BLOC], f16, tag="x16", name="x16")
                for blk in range(BLOC // 512):
                    bsl = slice(512 * blk, 512 * (blk + 1))
                    for k in range(KIN):
                        st = stagep.tile([128, 512], f32, tag="stage", name="stage")
                        nc.sync.dma_start(
                            out=st[:KC, :], in_=xT[KC * k:KC * (k + 1), bsl]
                        )
                        nc.vector.tensor_copy(
                            x16[:KC, BLOC * k + 512 * blk:BLOC * k + 512 * (blk + 1)],
                            st[:KC, :],
                        )
                    for c in range(HCH):
                        ps = spsum.tile([128, 512], f32, tag="spsum", name="spsum")
                        for k in range(KIN):
                            nc.tensor.matmul(
                                ps[:],
                                w1_16[:KC, HID * k + 128 * c:HID * k + 128 * (c + 1)],
                                x16[:KC, BLOC * k + 512 * blk:BLOC * k + 512 * (blk + 1)],
                                start=(k == 0),
                                stop=(k == KIN - 1),
                                tile_position=(0, 0),
                            )
                        nc.scalar.activation(
                            chv[c][:, 512 * blk:512 * (blk + 1)],
                            ps[:],
                            Act.Identity,
                            bias=b1c[c],
                            scale=1.0,
                        )

            # ---------- setup phase B: W2 forms, dbar2, y0, state0 ----------
            with (
                tc.tile_pool(name="stage2", bufs=2) as stage2p,
                tc.tile_pool(name="mst2", bufs=3) as mst2p,
                tc.tile_pool(name="spsum2", bufs=4, space="PSUM") as spsum2,
            ):
                # W2c = 0.5*W2 chunks [128, 4*10] (phi stationaries);
                # lives in the persistent const pool (used all loop)
                w2c16 = constp.tile([128, HCH * OUT], f16, tag="w2c16",
                                    name="w2c16")
                st = mst2p.tile([128, HCH * OUT], f32, tag="mst2", name="mst2")
                nc.sync.dma_start(
                    out=st.rearrange("p (c i) -> p c i", c=HCH),
                    in_=W2.rearrange("(c p) i -> p c i", c=HCH),
                )
                nc.vector.tensor_scalar_mul(w2c16[:], st[:], 0.5)
                W2c = [w2c16[:, OUT * c:OUT * (c + 1)] for c in range(HCH)]

                # W2Tr_c[32r+i, f] = 0.5*W2[128c+f, i], replicated to 4 row
                # groups via DMA broadcast of the transposed slice.
                for c in range(HCH):
                    st = mst2p.tile([128, 128], f32, tag="mst2b", name="mst2b")
                    nc.vector.memset(st[:], 0.0)
                    for r in range(NBLK):
                        nc.sync.dma_start(
                            out=st[32 * r:32 * r + OUT, :],
                            in_=W2[128 * c:128 * (c + 1), :].rearrange("m i -> i m"),
                        )
                    nc.vector.tensor_scalar_mul(W2Tr[c], st[:], 0.5)

                # ublk2 = -(c_h@W2) over the s-chunks only (phi streams s
                # there; h'-chunks stream h' so no correction needed).
                # zero fully: dbar2 is STREAMED through a PE matmul, where
                # 0 x Inf from garbage rows would poison the accumulation
                ublk2 = mst2p.tile([128, 1024], f32, tag="ublk2", name="ublk2",
                                   bufs=1)
                nc.vector.memset(ublk2[:], 0.0)
                for half in range(2 * NBLK):
                    j, hf = half // 2, half % 2
                    ps = spsum2.tile([128, 512], f32, tag="spsum2", name="spsum2")
                    for ci, c in enumerate(SC):
                        nc.tensor.matmul(
                            ps[32 * j:32 * j + OUT, :],
                            W2c[c],
                            chv[c][:, 1024 * j + 512 * hf:1024 * j + 512 * (hf + 1)],
                            start=(ci == 0),
                            stop=(ci == len(SC) - 1),
                            tile_position=(0, 32 * j),
                        )
                    nc.scalar.activation(
                        ublk2[32 * j:32 * j + OUT, 512 * hf:512 * (hf + 1)],
                        ps[32 * j:32 * j + OUT, :],
                        Act.Identity,
                        bias=0.0,
                        scale=-2.0,
                    )

                # b2 replicated to rows 32j+i as a per-partition column (x1.0
                # since dbar2 = 2*dbar)
                stb = mst2p.tile([128, 1], f32, tag="b2st", name="b2st", bufs=1)
                nc.vector.memset(stb[:], 0.0)
                for j in range(NBLK):
                    nc.sync.dma_start(out=stb[32 * j:32 * j + OUT, 0:1], in_=b2[:])

                # dbar2 = 0.5*onehot + b2 - (c_h@W2)  (= 2*dbar).  tgt lands
                # on rows {0,32,64,96}; R2 matmul broadcasts to 32-row groups.
                t32 = mst2p.tile([128, 1024], i32, tag="mst3", name="mst3")
                nc.vector.memset(t32[:], 0)
                for j in range(NBLK):
                    nc.sync.dma_start(
                        out=t32[32 * j:32 * j + 1, :],
                        in_=tgt[1024 * j:1024 * (j + 1), 0:1].rearrange("a b -> b a"),
                    )
                tf = mst2p.tile([128, 1024], f32, tag="mst3", name="mst3")
                nc.vector.tensor_copy(tf[:], t32[:])
                tf16 = mst2p.tile([128, 1024], f16, tag="mst3", name="mst3")
                nc.vector.tensor_copy(tf16[:], tf[:])
                eq = mst2p.tile([128, 1024], f32, tag="eqt", name="eqt", bufs=1)
                for hf in range(2):
                    ps = spsum2.tile([128, 512], f32, tag="spsum2", name="spsum2")
                    nc.tensor.matmul(
                        ps[:], R2_t, tf16[:, 512 * hf:512 * (hf + 1)],
                        start=True, stop=True, tile_position=(0, 0),
                    )
                    nc.vector.tensor_scalar(
                        eq[:, 512 * hf:512 * (hf + 1)], ps[:],
                        idxf_t, 0.5, Alu.is_equal, Alu.mult,
                    )
                eq2 = mst2p.tile([128, 1024], f32, tag="eq2", name="eq2", bufs=1)
                nc.vector.tensor_scalar(eq2[:], eq[:], stb, 0.0, Alu.add, Alu.add)
                nc.vector.tensor_tensor(dbar2[:], eq2[:], ublk2[:], Alu.add)

                # y0 blocked
                yst = mst2p.tile([128, 1024], f32, tag="mst3", name="mst3")
                nc.vector.memset(yst[:], 0.0)
                for j in range(NBLK):
                    nc.sync.dma_start(
                        out=yst[32 * j:32 * j + OUT, :],
                        in_=yT[:, 1024 * j:1024 * (j + 1)],
                    )
                ycur = yp.tile([128, 1024], f16, tag="yblk", name="yblk")
                nc.vector.tensor_copy(ycur[:], yst[:])

                # state0: chunks 0,1 -> h'0 = h0; chunks 2,3 -> s0 = h0 + c_h
                s0 = sp.tile([128, HCH * BLOC], f16, tag="s", name="s")
                for c in range(HCH):
                    st = stage2p.tile([128, BLOC], f32, tag="stage2", name="stage2")
                    nc.sync.dma_start(out=st[:], in_=hT[128 * c:128 * (c + 1), :])
                    if c in R1C:
                        nc.vector.tensor_copy(
                            s0[:, BLOC * c:BLOC * (c + 1)], st[:]
                        )
                    else:
                        nc.vector.tensor_tensor(
                            s0[:, BLOC * c:BLOC * (c + 1)], st[:], chv[c][:], Alu.add
                        )
                scur = s0

            # ---------- relaxation loop ----------
            with (
                tc.tile_pool(name="pu", bufs=3, space="PSUM") as pup,
                tc.tile_pool(name="py", bufs=1, space="PSUM") as pyp,
                tc.tile_pool(name="tmp", bufs=4) as tmpp,
                tc.tile_pool(name="hout", bufs=2) as houtp,
            ):
                blocks = [(c, m) for c in range(HCH) for m in CHUNK_MS[c]]
                # DVE-route pairs, spread mid/end so ACT load stays smooth:
                R1D = {(1, 3), (1, 2)}   # h'-state, DVE clip straight to state
                R4P = {(3, 0), (3, 1)}   # s-state, DVE clip + pair add
                for t in range(T):
                    last = t == T - 1
                    sv = [scur[:, BLOC * c:BLOC * (c + 1)] for c in range(HCH)]
                    hc = None
                    if last:
                        snext = None
                    else:
                        snext = sp.tile([128, HCH * BLOC], f16, tag="s", name="s")

                    py = pyp.tile([128, 1024], f32, tag="py", name="py")
                    pend = {}
                    waves = [blocks[i:i + 3] for i in range(0, len(blocks), 3)]
                    wbase = 0
                    for iw, wave in enumerate(waves):
                      pus = []
                      # identity streams (1024-col matmuls), back-to-back
                      for c, m in wave:
                        pu = pup.tile([128, 1024], f32, tag="pu", name="pu")
                        pus.append(pu)
                        for hf in range(2):
                            cs = slice(1024 * m + 512 * hf,
                                       1024 * m + 512 * (hf + 1))
                            ps = slice(512 * hf, 512 * (hf + 1))
                            nc.tensor.matmul(
                                pu[:, ps], halfI_t, sv[c][:, cs],
                                start=True, stop=False, tile_position=(0, 0),
                            )
                            if c in R1C:
                                nc.tensor.matmul(
                                    pu[:, ps], halfI_t, chv[c][:, cs],
                                    start=False, stop=False,
                                    tile_position=(0, 0),
                                )
                      # psi for the wave: adjacent matmuls on distinct row
                      # groups (per-strip LDW) -> concurrent
                      for hf in range(2):
                        for (c, m), pu in zip(wave, pus):
                            nc.tensor.matmul(
                                pu[:, 512 * hf:512 * (hf + 1)],
                                W2Tr[c][32 * m:32 * m + OUT, :],
                                ycur[32 * m:32 * m + OUT,
                                     512 * hf:512 * (hf + 1)],
                                start=False, stop=True,
                                tile_position=(32 * m, 0),
                            )
                      # after wave 0: the whole y path (phi reads scur, valid
                      # all step).  PE chews this while ACT/DVE catch up on
                      # wave 0, and y is ready long before step t+1's psi.
                      if iw == 0:
                        for hf in range(2):
                            psl = slice(512 * hf, 512 * (hf + 1))
                            for c in range(HCH):
                                for j in range(NBLK):
                                    nc.tensor.matmul(
                                        py[32 * j:32 * j + OUT, psl],
                                        W2c[c],
                                        sv[c][:, 1024 * j + 512 * hf:
                                              1024 * j + 512 * (hf + 1)],
                                        start=(c == 0), stop=False,
                                        tile_position=(0, 32 * j),
                                        skip_group_check=True,
                                    )
                            for j in range(NBLK):
                                nc.tensor.matmul(
                                    py[32 * j:32 * j + OUT, psl],
                                    I10q_t[32 * j:32 * j + OUT, :],
                                    ycur[32 * j:32 * j + OUT, psl],
                                    start=False, stop=False,
                                    tile_position=(32 * j, 32 * j),
                                    skip_group_check=True,
                                )
                            # dbar2 = 2*dbar streamed through 0.5*I128 (fills
                            # all 128 rows so the y-clip reads no stale PSUM)
                            nc.tensor.matmul(
                                py[:, psl], halfI_t, dbar2[:, psl],
                                start=False, stop=True,
                                tile_position=(0, 0),
                                skip_group_check=True,
                            )
                      for wi, ((c, m), pu) in enumerate(zip(wave, pus)):
                        w = wbase + wi
                        # ---- elementwise ----
                        half = m % 2
                        mcols = slice(1024 * m, 1024 * (m + 1))
                        scols = slice(BLOC * c + 1024 * m, BLOC * c + 1024 * (m + 1))
                        if last:
                            # final step: emit h (fp32) per chunk, DMA as
                            # each chunk completes; split ACT/DVE evenly
                            if w % NBLK == 0:
                                hc = houtp.tile([128, BLOC], f32, tag="hout",
                                                name="hout", bufs=2)
                            if (c, m) in R4P or (c, m) in R1D or c == 2:
                                nc.vector.tensor_scalar(
                                    hc[:, mcols], pu[:],
                                    0.0, 1.0, Alu.max, Alu.min,
                                )
                            else:
                                r = tmpp.tile([128, 1024], f32, tag="lr",
                                              name="lr", bufs=2)
                                nc.scalar.activation(r[:], pu[:], Act.Relu)
                                nc.vector.tensor_scalar(
                                    hc[:, mcols], r[:],
                                    1.0, 0.0, Alu.min, Alu.max,
                                )
                            if w % NBLK == NBLK - 1:
                                nc.sync.dma_start(
                                    out=hT_out[128 * c:128 * (c + 1), :],
                                    in_=hc[:],
                                )
                        elif (c, m) in R1D:
                            # h'-state, DVE-only: clip from PSUM to state
                            nc.vector.tensor_scalar(
                                snext[:, scols], pu[:],
                                0.0, 1.0, Alu.max, Alu.min,
                            )
                        elif (c, m) in R4P:
                            # s-state, DVE-only: clip from PSUM; pair-add c_h
                            key = (c, m // 2, "r4")
                            done = key in pend
                            if done:
                                rpair = pend.pop(key)
                            else:
                                rpair = tmpp.tile([128, 2048], f16,
                                                  tag="r4", name="r4", bufs=2)
                                pend[key] = rpair
                            nc.vector.tensor_scalar(
                                rpair[:, 1024 * half:1024 * (half + 1)], pu[:],
                                0.0, 1.0, Alu.max, Alu.min,
                            )
                            if done:
                                pc = slice(BLOC * c + 2048 * (m // 2),
                                           BLOC * c + 2048 * (m // 2 + 1))
                                cc = slice(2048 * (m // 2), 2048 * (m // 2 + 1))
                                nc.vector.tensor_tensor(
                                    snext[:, pc], rpair[:], chv[c][:, cc], Alu.add
                                )
                        else:
                            # ACT relu into the pair tile
                            key = (c, m // 2, "rp")
                            done = key in pend
                            if done:
                                rpair = pend.pop(key)
                            else:
                                rpair = tmpp.tile([128, 2048], f16,
                                                  tag="rp", name="rp", bufs=3)
                                pend[key] = rpair
                            nc.scalar.activation(
                                rpair[:, 1024 * half:1024 * (half + 1)], pu[:],
                                Act.Relu,
                            )
                            if done:
                                pc = slice(BLOC * c + 2048 * (m // 2),
                                           BLOC * c + 2048 * (m // 2 + 1))
                                cc = slice(2048 * (m // 2), 2048 * (m // 2 + 1))
                                if c in R1C:
                                    # state h': just cap at 1 (4x-mode DVE)
                                    nc.vector.tensor_scalar(
                                        snext[:, pc], rpair[:],
                                        1.0, 0.0, Alu.min, Alu.max,
                                    )
                                else:
                                    # state s: 4x-mode min then 2x-mode add
                                    nc.vector.tensor_scalar(
                                        rpair[:], rpair[:],
                                        1.0, 0.0, Alu.min, Alu.max,
                                    )
                                    nc.vector.tensor_tensor(
                                        snext[:, pc], rpair[:], chv[c][:, cc],
                                        Alu.add,
                                    )
                      wbase += len(wave)
                      # y-clip after wave 1: py complete well before this,
                      # and ynext lands early enough for step t+1's psi
                      if iw == 1:
                        ynext = yp.tile([128, 1024], f16, tag="yblk",
                                        name="yblk")
                        nc.vector.tensor_scalar(
                            ynext[:], py[:], 0.0, 1.0, Alu.max, Alu.min
                        )

                    if not last:
                        scur = snext
                    ycur = ynext

                # ---------- tail ----------
                yst2 = tmpp.tile([128, 1024], f32, tag="yo", name="yo", bufs=1)
                nc.vector.tensor_copy(yst2[:], ycur[:])
                for j in range(NBLK):
                    nc.sync.dma_start(
                        out=yT_out[:, 1024 * j:1024 * (j + 1)],
                        in_=yst2[32 * j:32 * j + OUT, :],
                    )

    if not nc.is_finalized():
        nc.finalize()
    return nc


def _consts():
    cst16 = np.zeros((128, CF16_W), dtype=np.float16)
    cst16[:, C_HALFI:C_HALFI + 128] = 0.5 * np.eye(128, dtype=np.float16)
    cst32 = np.zeros((128, 8), dtype=np.float32)
    cst32[:, 0] = -1.0
    for j in range(NBLK):
        for i in range(OUT):
            cst16[32 * j + i, C_I10Q + i] = 0.25
            cst16[32 * j, C_R2 + 32 * j + i] = 1.0
            cst32[32 * j + i, 0] = float(i)
    return cst16, cst32


def prepare(inputs):
    x = np.asarray(inputs["x"], dtype=np.float32)
    h0 = np.asarray(inputs["h_init"], dtype=np.float32)
    y0 = np.asarray(inputs["y_init"], dtype=np.float32)
    W1 = np.ascontiguousarray(np.asarray(inputs["W1"], dtype=np.float32))
    W2 = np.ascontiguousarray(np.asarray(inputs["W2"], dtype=np.float32))
    b1 = np.ascontiguousarray(
        np.asarray(inputs["b1"], dtype=np.float32).reshape(HID, 1)
    )
    b2 = np.ascontiguousarray(
        np.asarray(inputs["b2"], dtype=np.float32).reshape(OUT, 1)
    )
    target = np.ascontiguousarray(inputs["target"])
    T = int(inputs["T"])

    xT = np.ascontiguousarray(x.T)      # [IN, B]
    hT = np.ascontiguousarray(h0.T)     # [HID, B]
    yT = np.ascontiguousarray(y0.T)     # [OUT, B]
    if target.dtype == np.int64:
        tgt32 = target.view(np.int32).reshape(B, 2)  # int64 -> (lo, hi) pairs
    else:
        tgt32 = np.zeros((B, 2), dtype=np.int32)
        tgt32[:, 0] = target

    key = T
    if key not in _BUILT:
        _BUILT[key] = _build(T)
    nc = _BUILT[key]

    cst16, cst32 = _consts()
    in_maps = []
    for k in range(N_CORES):
        sl = slice(k * BLOC, (k + 1) * BLOC)
        in_maps.append({
            "xT": np.ascontiguousarray(xT[:, sl]),
            "hT": np.ascontiguousarray(hT[:, sl]),
            "yT": np.ascontiguousarray(yT[:, sl]),
            "W1": W1, "W2": W2, "b1": b1, "b2": b2,
            "tgt": np.ascontiguousarray(tgt32[sl]),
            "cst16": cst16, "cst32": cst32,
        })

    return nc, in_maps


def assemble(results):
    out = np.empty((B, HID + OUT), dtype=np.float32)
    for k in range(N_CORES):
        sl = slice(k * BLOC, (k + 1) * BLOC)
        out[sl, :HID] = np.asarray(results[k]["hT_out"]).T
        out[sl, HID:] = np.asarray(results[k]["yT_out"]).T
    return out


def kernel(**inputs):
    from concourse import bass_utils

    nc, in_maps = prepare(inputs)
    res = bass_utils.run_bass_kernel_spmd(nc, in_maps, list(range(N_CORES)))
    globals()["_LAST_RESULTS"] = res
    return assemble(res.results)


# revision 27
# speedup vs baseline: 1.4929x; 1.0153x over previous
# Trainium2 Bass kernel for nn_EqPropNetwork (equilibrium-propagation relaxation).
#
# Math (per reference.py):
#   c_h = x @ W1 + b1                                  [B, HID]  (constant over steps)
#   repeat T times:
#     psi = y @ W2.T ; phi = h @ W2
#     h'  = clip(0.5*h + 0.5*c_h + 0.5*psi, 0, 1)
#     y'  = clip(0.25*y + 0.5*phi + 0.5*b2 + 0.25*onehot(target), 0, 1)
#   out = concat(h, y)                                  [B, HID+OUT]
#
# Mapping (per core, B_loc = 4096, pure data parallel over 8 cores):
#   Feature-major state: partition = feature, free = batch; 4 chunks x 4
#   batch-blocks of [128, 1024].  Engine-balanced routes per chunk:
#   * chunks 0,1 ("R1"): state = h' (post-clip).  PE identity streams BOTH
#     h' and c_h (PSUM u = 0.5h' + 0.5c_h + 0.5psi); ACT relu evacuates
#     PSUM; DVE finishes with a 4x-mode min/max.  (PE-side add is cheaper
#     than a DVE tensor_tensor add.)
#   * chunks 2,3: state = s := h + c_h.  PSUM u = 0.5s + 0.5psi.  Most
#     blocks: ACT relu + one DVE scalar_tensor_tensor (min,add); one pair
#     runs DVE-only (clip from PSUM + 2x tensor_tensor add) to offload ACT.
#   * y-update rides PE: py = 0.5*(s@W2) + 0.25*y (diag-tiled I10 matmuls)
#     + dbar (identity matmul streaming dbar2 = 2*dbar), then a single DVE
#     clip.  dbar folds onehot/4 + b2/2 - 0.5*(c_h@W2) over the s-chunks.
#   * psi matmuls: per-strip LDWEIGHTS (rows 32m..32m+10 only) and block
#     order rotates the batch-block m so adjacent psi matmuls land on
#     distinct PE row groups and overlap.  phi matmuls are 4-way
#     col-group packed.
import sys

import numpy as np

if "/opt/trn_rl_repo" not in sys.path:
    sys.path.insert(0, "/opt/trn_rl_repo")

N_CORES = 8
B, IN, HID, OUT = 32768, 784, 512, 10
BLOC = B // N_CORES  # 4096
NBLK = BLOC // 1024  # 4 batch blocks of 1024
KIN = 7              # IN chunks of 112
KC = IN // KIN       # 112
HCH = HID // 128     # 4 hidden chunks

R1C = (0, 1)         # chunks with h'-state (PE adds c_h)
SC = (2, 3)          # chunks with s-state

# packed fp16 const tile column offsets
C_HALFI = 0          # [128, 128] 0.5*I128
C_W2TR = 128         # 4 x [128, 128] W2T replicated, pre-scaled 0.5
C_I10Q = 640         # [128, 10] 0.25*I10 at 4 row offsets
C_R2 = 650           # [128, 128] rep: R2[32j, 32j+i]=1 (i<10) -> bcast block rows
CF16_W = 778

# per-chunk block (batch-block m) orders: chosen so waves of 3 consecutive
# blocks get distinct m (psi row-group overlap) while pair halves stay in
# the same 2048-col group (m//2) for paired DVE ops
CHUNK_MS = {0: (0, 1, 2, 3), 1: (1, 0, 3, 2), 2: (1, 0, 3, 2), 3: (3, 2, 0, 1)}

_BUILT = {}


def _build(T):
    import concourse.bass as bass
    from concourse import bacc, mybir
    from concourse.tile import TileContext

    f32 = mybir.dt.float32
    f16 = mybir.dt.float16
    i32 = mybir.dt.int32
    Alu = mybir.AluOpType
    Act = mybir.ActivationFunctionType

    nc = bacc.Bacc("TRN2", target_bir_lowering=False)

    xT = nc.declare_dram_parameter("xT", [IN, BLOC], f32, isOutput=False)
    hT = nc.declare_dram_parameter("hT", [HID, BLOC], f32, isOutput=False)
    yT = nc.declare_dram_parameter("yT", [OUT, BLOC], f32, isOutput=False)
    W1 = nc.declare_dram_parameter("W1", [IN, HID], f32, isOutput=False)
    W2 = nc.declare_dram_parameter("W2", [HID, OUT], f32, isOutput=False)
    b1 = nc.declare_dram_parameter("b1", [HID, 1], f32, isOutput=False)
    b2 = nc.declare_dram_parameter("b2", [OUT, 1], f32, isOutput=False)
    tgt = nc.declare_dram_parameter("tgt", [BLOC, 2], i32, isOutput=False)
    cst16 = nc.declare_dram_parameter("cst16", [128, CF16_W], f16, isOutput=False)
    cst32 = nc.declare_dram_parameter("cst32", [128, 8], f32, isOutput=False)

    hT_out = nc.declare_dram_parameter("hT_out", [HID, BLOC], f32, isOutput=True)
    yT_out = nc.declare_dram_parameter("yT_out", [OUT, BLOC], f32, isOutput=True)

    with TileContext(nc) as tc:
        with (
            tc.tile_pool(name="const", bufs=1) as constp,
            tc.tile_pool(name="ch", bufs=1) as chp,
            tc.tile_pool(name="state", bufs=2) as sp,
            tc.tile_pool(name="ypool", bufs=2) as yp,
        ):
            cf16 = constp.tile([128, CF16_W], f16, tag="cf16", name="cf16")
            cf32 = constp.tile([128, 8], f32, tag="cf32", name="cf32")
            dbar2 = constp.tile([128, 1024], f16, tag="dbar2", name="dbar2")
            cb1 = constp.tile([128, HCH], f32, tag="cb1", name="cb1")
            nc.sync.dma_start(out=cf16[:], in_=cst16[:])
            nc.sync.dma_start(out=cf32[:], in_=cst32[:])
            nc.sync.dma_start(
                out=cb1.rearrange("p (c o) -> p c o", c=HCH),
                in_=b1.rearrange("(c p) o -> p c o", c=HCH),
            )

            halfI_t = cf16[:, C_HALFI:C_HALFI + 128]
            W2Tr = [cf16[:, C_W2TR + 128 * c:C_W2TR + 128 * (c + 1)]
                    for c in range(HCH)]
            I10q_t = cf16[:, C_I10Q:C_I10Q + OUT]
            idxf_t = cf32[:, 0:1]
            b1c = [cb1[:, c:c + 1] for c in range(HCH)]
            R2_t = cf16[:, C_R2:C_R2 + 128]

            ch = chp.tile([128, HCH * BLOC], f16, tag="ch", name="ch")
            chv = [ch[:, BLOC * c:BLOC * (c + 1)] for c in range(HCH)]

            # ---------- setup phase A: c_h = x@W1 + b1 ----------
            # x loaded in 1024-col batch stripes (4 KiB/partition lines),
            # 128-row contraction chunks (6x128 + 16), with the stripe DMAs
            # spread across the sync and gpsimd queues so transfers overlap.
            CH_K = [(128 * k, min(128, IN - 128 * k)) for k in range((IN + 127) // 128)]
            NKC = len(CH_K)
            with (
                tc.tile_pool(name="x16p", bufs=1) as x16p,
                tc.tile_pool(name="stage", bufs=6) as stagep,
                tc.tile_pool(name="mst", bufs=3) as mstp,
                tc.tile_pool(name="spsum", bufs=4, space="PSUM") as spsum,
            ):
                w1_16 = x16p.tile([128, NKC * HID], f16, tag="w1_16", name="w1_16")
                for k, (ko, kn) in enumerate(CH_K):
                    st = mstp.tile([128, HID], f32, tag="mst", name="mst")
                    nc.sync.dma_start(out=st[:kn, :], in_=W1[ko:ko + kn, :])
                    nc.vector.tensor_copy(
                        w1_16[:kn, HID * k:HID * (k + 1)], st[:kn, :]
                    )
                x16 = x16p.tile([128, NKC * BLOC], f16, tag="x16", name="x16")
                for blk in range(BLOC // 1024):
                    bsl = slice(1024 * blk, 1024 * (blk + 1))
                    for k, (ko, kn) in enumerate(CH_K):
                        st = stagep.tile([128, 1024], f32, tag="stage", name="stage")
                        nc.sync.dma_start(out=st[:kn, :], in_=xT[ko:ko + kn, bsl])
                        nc.vector.tensor_copy(
                            x16[:kn, BLOC * k + 1024 * blk:
                                BLOC * k + 1024 * (blk + 1)],
                            st[:kn, :],
                        )
                    for hf in range(2):
                        for c in range(HCH):
                            ps = spsum.tile([128, 512], f32, tag="spsum",
                                            name="spsum")
                            for k, (ko, kn) in enumerate(CH_K):
                                nc.tensor.matmul(
                                    ps[:],
                                    w1_16[:kn, HID * k + 128 * c:
                                          HID * k + 128 * (c + 1)],
                                    x16[:kn, BLOC * k + 1024 * blk + 512 * hf:
                                        BLOC * k + 1024 * blk + 512 * (hf + 1)],
                                    start=(k == 0),
                                    stop=(k == NKC - 1),
                                    tile_position=(0, 0),
                                )
                            nc.scalar.activation(
                                chv[c][:, 1024 * blk + 512 * hf:
                                       1024 * blk + 512 * (hf + 1)],
                                ps[:],
                                Act.Identity,
                                bias=b1c[c],
                                scale=1.0,
                            )

            # ---------- setup phase B: W2 forms, dbar2, y0, state0 ----------
            with (
                tc.tile_pool(name="stage2", bufs=2) as stage2p,
                tc.tile_pool(name="mst2", bufs=3) as mst2p,
                tc.tile_pool(name="spsum2", bufs=4, space="PSUM") as spsum2,
            ):
                # W2c = 0.5*W2 chunks [128, 4*10] (phi stationaries);
                # lives in the persistent const pool (used all loop)
                w2c16 = constp.tile([128, HCH * OUT], f16, tag="w2c16",
                                    name="w2c16")
                st = mst2p.tile([128, HCH * OUT], f32, tag="mst2", name="mst2")
                nc.sync.dma_start(
                    out=st.rearrange("p (c i) -> p c i", c=HCH),
                    in_=W2.rearrange("(c p) i -> p c i", c=HCH),
                )
                nc.vector.tensor_scalar_mul(w2c16[:], st[:], 0.5)
                W2c = [w2c16[:, OUT * c:OUT * (c + 1)] for c in range(HCH)]

                # W2Tr_c[32r+i, f] = 0.5*W2[128c+f, i], replicated to 4 row
                # groups via DMA broadcast of the transposed slice.
                for c in range(HCH):
                    st = mst2p.tile([128, 128], f32, tag="mst2b", name="mst2b")
                    nc.vector.memset(st[:], 0.0)
                    for r in range(NBLK):
                        nc.sync.dma_start(
                            out=st[32 * r:32 * r + OUT, :],
                            in_=W2[128 * c:128 * (c + 1), :].rearrange("m i -> i m"),
                        )
                    nc.vector.tensor_scalar_mul(W2Tr[c], st[:], 0.5)

                # ublk2 = -(c_h@W2) over the s-chunks only (phi streams s
                # there; h'-chunks stream h' so no correction needed).
                # zero fully: dbar2 is STREAMED through a PE matmul, where
                # 0 x Inf from garbage rows would poison the accumulation
                ublk2 = mst2p.tile([128, 1024], f32, tag="ublk2", name="ublk2",
                                   bufs=1)
                nc.vector.memset(ublk2[:], 0.0)
                for half in range(2 * NBLK):
                    j, hf = half // 2, half % 2
                    ps = spsum2.tile([128, 512], f32, tag="spsum2", name="spsum2")
                    for ci, c in enumerate(SC):
                        nc.tensor.matmul(
                            ps[32 * j:32 * j + OUT, :],
                            W2c[c],
                            chv[c][:, 1024 * j + 512 * hf:1024 * j + 512 * (hf + 1)],
                            start=(ci == 0),
                            stop=(ci == len(SC) - 1),
                            tile_position=(0, 32 * j),
                        )
                    nc.scalar.activation(
                        ublk2[32 * j:32 * j + OUT, 512 * hf:512 * (hf + 1)],
                        ps[32 * j:32 * j + OUT, :],
                        Act.Identity,
                        bias=0.0,
                        scale=-2.0,
                    )

                # b2 replicated to rows 32j+i as a per-partition column (x1.0
                # since dbar2 = 2*dbar)
                stb = mst2p.tile([128, 1], f32, tag="b2st", name="b2st", bufs=1)
                nc.vector.memset(stb[:], 0.0)
                for j in range(NBLK):
                    nc.sync.dma_start(out=stb[32 * j:32 * j + OUT, 0:1], in_=b2[:])

                # dbar2 = 0.5*onehot + b2 - (c_h@W2)  (= 2*dbar).  tgt lands
                # on rows {0,32,64,96}; R2 matmul broadcasts to 32-row groups.
                t32 = mst2p.tile([128, 1024], i32, tag="mst3", name="mst3")
                nc.vector.memset(t32[:], 0)
                for j in range(NBLK):
                    nc.sync.dma_start(
                        out=t32[32 * j:32 * j + 1, :],
                        in_=tgt[1024 * j:1024 * (j + 1), 0:1].rearrange("a b -> b a"),
                    )
                tf = mst2p.tile([128, 1024], f32, tag="mst3", name="mst3")
                nc.vector.tensor_copy(tf[:], t32[:])
                tf16 = mst2p.tile([128, 1024], f16, tag="mst3", name="mst3")
                nc.vector.tensor_copy(tf16[:], tf[:])
                eq = mst2p.tile([128, 1024], f32, tag="eqt", name="eqt", bufs=1)
                for hf in range(2):
                    ps = spsum2.tile([128, 512], f32, tag="spsum2", name="spsum2")
                    nc.tensor.matmul(
                        ps[:], R2_t, tf16[:, 512 * hf:512 * (hf + 1)],
                        start=True, stop=True, tile_position=(0, 0),
                    )
                    nc.vector.tensor_scalar(
                        eq[:, 512 * hf:512 * (hf + 1)], ps[:],
                        idxf_t, 0.5, Alu.is_equal, Alu.mult,
                    )
                eq2 = mst2p.tile([128, 1024], f32, tag="eq2", name="eq2", bufs=1)
                nc.vector.tensor_scalar(eq2[:], eq[:], stb, 0.0, Alu.add, Alu.add)
                nc.vector.tensor_tensor(dbar2[:], eq2[:], ublk2[:], Alu.add)

                # y0 blocked
                yst = mst2p.tile([128, 1024], f32, tag="mst3", name="mst3")
                nc.vector.memset(yst[:], 0.0)
                for j in range(NBLK):
                    nc.sync.dma_start(
                        out=yst[32 * j:32 * j + OUT, :],
                        in_=yT[:, 1024 * j:1024 * (j + 1)],
                    )
                ycur = yp.tile([128, 1024], f16, tag="yblk", name="yblk")
                nc.vector.tensor_copy(ycur[:], yst[:])

                # state0: chunks 0,1 -> h'0 = h0; chunks 2,3 -> s0 = h0 + c_h
                s0 = sp.tile([128, HCH * BLOC], f16, tag="s", name="s")
                for c in range(HCH):
                    st = stage2p.tile([128, BLOC], f32, tag="stage2", name="stage2")
                    nc.sync.dma_start(out=st[:], in_=hT[128 * c:128 * (c + 1), :])
                    if c in R1C:
                        nc.vector.tensor_copy(
                            s0[:, BLOC * c:BLOC * (c + 1)], st[:]
                        )
                    else:
                        nc.vector.tensor_tensor(
                            s0[:, BLOC * c:BLOC * (c + 1)], st[:], chv[c][:], Alu.add
                        )
                scur = s0

            # ---------- relaxation loop ----------
            with (
                tc.tile_pool(name="pu", bufs=3, space="PSUM") as pup,
                tc.tile_pool(name="py", bufs=1, space="PSUM") as pyp,
                tc.tile_pool(name="tmp", bufs=4) as tmpp,
                tc.tile_pool(name="hout", bufs=2) as houtp,
            ):
                blocks = [(c, m) for c in range(HCH) for m in CHUNK_MS[c]]
                # DVE-route pairs, spread mid/end so ACT load stays smooth:
                R1D = {(1, 3), (1, 2)}   # h'-state, DVE clip straight to state
                R4P = {(3, 0), (3, 1)}   # s-state, DVE clip + pair add
                for t in range(T):
                    last = t == T - 1
                    sv = [scur[:, BLOC * c:BLOC * (c + 1)] for c in range(HCH)]
                    hc = None
                    if last:
                        snext = None
                    else:
                        snext = sp.tile([128, HCH * BLOC], f16, tag="s", name="s")

                    py = pyp.tile([128, 1024], f32, tag="py", name="py")
                    pend = {}
                    waves = [blocks[i:i + 3] for i in range(0, len(blocks), 3)]
                    wbase = 0
                    for iw, wave in enumerate(waves):
                      pus = []
                      # identity streams (1024-col matmuls), back-to-back
                      for c, m in wave:
                        pu = pup.tile([128, 1024], f32, tag="pu", name="pu")
                        pus.append(pu)
                        for hf in range(2):
                            cs = slice(1024 * m + 512 * hf,
                                       1024 * m + 512 * (hf + 1))
                            ps = slice(512 * hf, 512 * (hf + 1))
                            nc.tensor.matmul(
                                pu[:, ps], halfI_t, sv[c][:, cs],
                                start=True, stop=False, tile_position=(0, 0),
                            )
                            if c in R1C:
                                nc.tensor.matmul(
                                    pu[:, ps], halfI_t, chv[c][:, cs],
                                    start=False, stop=False,
                                    tile_position=(0, 0),
                                )
                      # psi for the wave: adjacent matmuls on distinct row
                      # groups (per-strip LDW) -> concurrent
                      for hf in range(2):
                        for (c, m), pu in zip(wave, pus):
                            nc.tensor.matmul(
                                pu[:, 512 * hf:512 * (hf + 1)],
                                W2Tr[c][32 * m:32 * m + OUT, :],
                                ycur[32 * m:32 * m + OUT,
                                     512 * hf:512 * (hf + 1)],
                                start=False, stop=True,
                                tile_position=(32 * m, 0),
                            )
                      # after wave 0: the whole y path (phi reads scur, valid
                      # all step).  PE chews this while ACT/DVE catch up on
                      # wave 0, and y is ready long before step t+1's psi.
                      if iw == 0:
                        for hf in range(2):
                            psl = slice(512 * hf, 512 * (hf + 1))
                            for c in range(HCH):
                                for j in range(NBLK):
                                    nc.tensor.matmul(
                                        py[32 * j:32 * j + OUT, psl],
                                        W2c[c],
                                        sv[c][:, 1024 * j + 512 * hf:
                                              1024 * j + 512 * (hf + 1)],
                                        start=(c == 0), stop=False,
                                        tile_position=(0, 32 * j),
                                        skip_group_check=True,
                                    )
                            for j in range(NBLK):
                                nc.tensor.matmul(
                                    py[32 * j:32 * j + OUT, psl],
                                    I10q_t[32 * j:32 * j + OUT, :],
                                    ycur[32 * j:32 * j + OUT, psl],
                                    start=False, stop=False,
                                    tile_position=(32 * j, 32 * j),
                                    skip_group_check=True,
                                )
                            # dbar2 = 2*dbar streamed through 0.5*I128 (fills
                            # all 128 rows so the y-clip reads no stale PSUM)
                            nc.tensor.matmul(
                                py[:, psl], halfI_t, dbar2[:, psl],
                                start=False, stop=True,
                                tile_position=(0, 0),
                                skip_group_check=True,
                            )
                      for wi, ((c, m), pu) in enumerate(zip(wave, pus)):
                        w = wbase + wi
                        # ---- elementwise ----
                        half = m % 2
                        mcols = slice(1024 * m, 1024 * (m + 1))
                        scols = slice(BLOC * c + 1024 * m, BLOC * c + 1024 * (m + 1))
                        if last:
                            # final step: emit h (fp32) per chunk, DMA as
                            # each chunk completes; split ACT/DVE evenly
                            if w % NBLK == 0:
                                hc = houtp.tile([128, BLOC], f32, tag="hout",
                                                name="hout", bufs=2)
                            if (c, m) in R4P or (c, m) in R1D or c == 2:
                                nc.vector.tensor_scalar(
                                    hc[:, mcols], pu[:],
                                    0.0, 1.0, Alu.max, Alu.min,
                                )
                            else:
                                r = tmpp.tile([128, 1024], f32, tag="lr",
                                              name="lr", bufs=2)
                                nc.scalar.activation(r[:], pu[:], Act.Relu)
                                nc.vector.tensor_scalar(
                                    hc[:, mcols], r[:],
                                    1.0, 0.0, Alu.min, Alu.max,
                                )
                            if w % NBLK == NBLK - 1:
                                nc.sync.dma_start(
                                    out=hT_out[128 * c:128 * (c + 1), :],
                                    in_=hc[:],
                                )
                        elif (c, m) in R1D:
                            # h'-state, DVE-only: clip from PSUM to state
                            nc.vector.tensor_scalar(
                                snext[:, scols], pu[:],
                                0.0, 1.0, Alu.max, Alu.min,
                            )
                        elif (c, m) in R4P:
                            # s-state, DVE-only: clip from PSUM; pair-add c_h
                            key = (c, m // 2, "r4")
                            done = key in pend
                            if done:
                                rpair = pend.pop(key)
                            else:
                                rpair = tmpp.tile([128, 2048], f16,
                                                  tag="r4", name="r4", bufs=2)
                                pend[key] = rpair
                            nc.vector.tensor_scalar(
                                rpair[:, 1024 * half:1024 * (half + 1)], pu[:],
                                0.0, 1.0, Alu.max, Alu.min,
                            )
                            if done:
                                pc = slice(BLOC * c + 2048 * (m // 2),
                                           BLOC * c + 2048 * (m // 2 + 1))
                                cc = slice(2048 * (m // 2), 2048 * (m // 2 + 1))
                                nc.vector.tensor_tensor(
                                    snext[:, pc], rpair[:], chv[c][:, cc], Alu.add
                                )
                        else:
                            # ACT relu into the pair tile
                            key = (c, m // 2, "rp")
                            done = key in pend
                            if done:
                                rpair = pend.pop(key)
                            else:
                                rpair = tmpp.tile([128, 2048], f16,
                                                  tag="rp", name="rp", bufs=3)
                                pend[key] = rpair
                            nc.scalar.activation(
                                rpair[:, 1024 * half:1024 * (half + 1)], pu[:],
                                Act.Relu,
                            )
                            if done:
                                pc = slice(BLOC * c + 2048 * (m // 2),
                                           BLOC * c + 2048 * (m // 2 + 1))
                                cc = slice(2048 * (m // 2), 2048 * (m // 2 + 1))
                                if c in R1C:
                                    # state h': just cap at 1 (4x-mode DVE)
                                    nc.vector.tensor_scalar(
                                        snext[:, pc], rpair[:],
                                        1.0, 0.0, Alu.min, Alu.max,
                                    )
                                else:
                                    # state s: 4x-mode min then 2x-mode add
                                    nc.vector.tensor_scalar(
                                        rpair[:], rpair[:],
                                        1.0, 0.0, Alu.min, Alu.max,
                                    )
                                    nc.vector.tensor_tensor(
                                        snext[:, pc], rpair[:], chv[c][:, cc],
                                        Alu.add,
                                    )
                      wbase += len(wave)
                      # y-clip after wave 1: py complete well before this,
                      # and ynext lands early enough for step t+1's psi
                      if iw == 1:
                        ynext = yp.tile([128, 1024], f16, tag="yblk",
                                        name="yblk")
                        nc.vector.tensor_scalar(
                            ynext[:], py[:], 0.0, 1.0, Alu.max, Alu.min
                        )

                    if not last:
                        scur = snext
                    ycur = ynext

                # ---------- tail ----------
                yst2 = tmpp.tile([128, 1024], f32, tag="yo", name="yo", bufs=1)
                nc.vector.tensor_copy(yst2[:], ycur[:])
                for j in range(NBLK):
                    nc.sync.dma_start(
                        out=yT_out[:, 1024 * j:1024 * (j + 1)],
                        in_=yst2[32 * j:32 * j + OUT, :],
                    )

    if not nc.is_finalized():
        nc.finalize()
    return nc


def _consts():
    cst16 = np.zeros((128, CF16_W), dtype=np.float16)
    cst16[:, C_HALFI:C_HALFI + 128] = 0.5 * np.eye(128, dtype=np.float16)
    cst32 = np.zeros((128, 8), dtype=np.float32)
    cst32[:, 0] = -1.0
    for j in range(NBLK):
        for i in range(OUT):
            cst16[32 * j + i, C_I10Q + i] = 0.25
            cst16[32 * j, C_R2 + 32 * j + i] = 1.0
            cst32[32 * j + i, 0] = float(i)
    return cst16, cst32


def prepare(inputs):
    x = np.asarray(inputs["x"], dtype=np.float32)
    h0 = np.asarray(inputs["h_init"], dtype=np.float32)
    y0 = np.asarray(inputs["y_init"], dtype=np.float32)
    W1 = np.ascontiguousarray(np.asarray(inputs["W1"], dtype=np.float32))
    W2 = np.ascontiguousarray(np.asarray(inputs["W2"], dtype=np.float32))
    b1 = np.ascontiguousarray(
        np.asarray(inputs["b1"], dtype=np.float32).reshape(HID, 1)
    )
    b2 = np.ascontiguousarray(
        np.asarray(inputs["b2"], dtype=np.float32).reshape(OUT, 1)
    )
    target = np.ascontiguousarray(inputs["target"])
    T = int(inputs["T"])

    xT = np.ascontiguousarray(x.T)      # [IN, B]
    hT = np.ascontiguousarray(h0.T)     # [HID, B]
    yT = np.ascontiguousarray(y0.T)     # [OUT, B]
    if target.dtype == np.int64:
        tgt32 = target.view(np.int32).reshape(B, 2)  # int64 -> (lo, hi) pairs
    else:
        tgt32 = np.zeros((B, 2), dtype=np.int32)
        tgt32[:, 0] = target

    key = T
    if key not in _BUILT:
        _BUILT[key] = _build(T)
    nc = _BUILT[key]

    cst16, cst32 = _consts()
    in_maps = []
    for k in range(N_CORES):
        sl = slice(k * BLOC, (k + 1) * BLOC)
        in_maps.append({
            "xT": np.ascontiguousarray(xT[:, sl]),
            "hT": np.ascontiguousarray(hT[:, sl]),
            "yT": np.ascontiguousarray(yT[:, sl]),
            "W1": W1, "W2": W2, "b1": b1, "b2": b2,
            "tgt": np.ascontiguousarray(tgt32[sl]),
            "cst16": cst16, "cst32": cst32,
        })

    return nc, in_maps


def assemble(results):
    out = np.empty((B, HID + OUT), dtype=np.float32)
    for k in range(N_CORES):
        sl = slice(k * BLOC, (k + 1) * BLOC)
        out[sl, :HID] = np.asarray(results[k]["hT_out"]).T
        out[sl, HID:] = np.asarray(results[k]["yT_out"]).T
    return out


def kernel(**inputs):
    from concourse import bass_utils

    nc, in_maps = prepare(inputs)
    res = bass_utils.run_bass_kernel_spmd(nc, in_maps, list(range(N_CORES)))
    globals()["_LAST_RESULTS"] = res
    return assemble(res.results)
